# revision 1
# baseline (speedup 1.0000x reference)
"""Trainium2 Bass kernel for the CudaNorm FastWeight DPFP transformer layer.

Sharding: batch (8) across the 8 cores; each core runs its batch's full layer:
qkvb projection, DPFP feature maps, chunked delta-rule fast-weight scan
(C=128, depth-1 Neumann solve), output projection, residual + LayerNorm.

Self-contained: hardcodes all shapes; host-side prep rearranges weights and
builds masks/identity constants passed as extra DRAM inputs.
"""
import os
import numpy as np
import ml_dtypes

import concourse.bass as bass
import concourse.mybir as mybir
from concourse.bass_utils import run_bass_kernel_spmd
from concourse.tile import TileContext
from concourse.vector_clock import ScopedClock, VectorClock
from contextlib import ExitStack

F32 = mybir.dt.float32
BF16 = mybir.dt.bfloat16
AF = mybir.ActivationFunctionType
OP = mybir.AluOpType
AX = mybir.AxisListType

SLEN, BSZ, DM = 2048, 8, 1024
NH, DH, NROLL = 16, 64, 2
D = 2 * NROLL * DH            # 256 feature dim
C = 128                       # chunk length
NCH = SLEN // C               # 16 chunks
EPS, LN_EPS = 1e-5, 1e-5
SCALE = 1.0 / float(np.sqrt(DH))
OQKV = NH * 192               # 3072
OTOT = OQKV + NH              # 3088 (qkv + per-head b columns)

# ---------------------------------------------------------------- tile ctx
MAXW = 2


class PatchedTileContext(TileContext):
    """Work around walrus TPB sync-command limits: each instruction carries at
    most 2 sync commands (waits+updates); hoist excess waits onto preceding
    same-engine NoOps (1 wait each), and emit the kernel-tail drain's waits
    one-per-nop on SP."""

    def _lower_ordered_insts(self, ordered):
        for bb_name in list(ordered.keys()):
            new = []
            for inst in ordered[bb_name]:
                si = inst.sync_info
                nupd = len(si.on_update) if si is not None and si.on_update else 0
                maxw = max(0, MAXW - nupd)
                if si is not None and si.on_wait and len(si.on_wait) > maxw:
                    waits = list(si.on_wait)
                    excess = waits if maxw == 0 else waits[:-maxw]
                    keep = [] if maxw == 0 else waits[-maxw:]
                    for w in excess:
                        nop = mybir.InstNoOp(
                            name=self.nc.get_next_instruction_name(),
                            engine=inst.engine, ins=[], outs=[])
                        nop.sync_info = mybir.SyncInfo(on_wait=[w], on_update=[])
                        new.append(nop)
                    inst.sync_info = mybir.SyncInfo(
                        on_wait=keep, on_update=list(si.on_update or []))
                new.append(inst)
            ordered[bb_name] = new
        return super()._lower_ordered_insts(ordered)

    def _drain_and_barrier(self, tick_clock, wait_clock):
        gc = tick_clock.global_clock
        n = len(gc)
        for p in range(n):
            if gc[p] > 0:
                vc = VectorClock([gc[i] if i == p else 0 for i in range(n)])
                nop = self.nc.sync.nop(nofuse=True)
                wait_clock.add_sem_waits(nop.ins, ScopedClock({None: vc}))
        self.nc.sync.drain()
        self.nc.all_engine_barrier()
        assert self.sems is not None
        popped = self.nc._tile_sem_poison_stack.pop()
        assert popped is self._sem_poison
        self.nc.clear_and_free_semaphores(list(self.sems.allocated().values()))
        self.nc.all_engine_barrier()


# ---------------------------------------------------------------- program
def build_program(n_chunks=NCH, n_heads=NH):
    nc = bass.Bass()
    d_hT = nc.declare_dram_parameter("hT", [DM, SLEN], BF16, isOutput=False)
    d_hres = nc.declare_dram_parameter("hres", [SLEN, DM], F32, isOutput=False)
    d_w = nc.declare_dram_parameter("wqkv", [DM, OTOT], BF16, isOutput=False)
    d_wo = nc.declare_dram_parameter("woT", [DM, DM], BF16, isOutput=False)
    d_lng = nc.declare_dram_parameter("lng", [128, DM], F32, isOutput=False)
    d_lnb = nc.declare_dram_parameter("lnb", [128, DM], F32, isOutput=False)
    d_mSL = nc.declare_dram_parameter("maskSL", [128, 132], F32, isOutput=False)
    d_mLI = nc.declare_dram_parameter("maskLI", [128, 132], F32, isOutput=False)
    d_mUI = nc.declare_dram_parameter("maskUI", [128, 132], F32, isOutput=False)
    d_id = nc.declare_dram_parameter("identb", [128, 128], BF16, isOutput=False)
    d_out = nc.declare_dram_parameter("out", [SLEN, DM], F32, isOutput=True)

    with PatchedTileContext(nc) as tc, ExitStack() as ctx:
        # ---- persistent pools (bufs=1)
        P = lambda name, bufs, **kw: ctx.enter_context(
            tc.tile_pool(name=name, bufs=bufs, **kw))
        const = P("const", 1)
        state = P("state", 1)
        # ---- streaming pools
        import os as _os
        B = lambda k, d: int(_os.environ.get(k, d))
        hts_p = P("hts", 2)
        raw_p = P("raw", 2)
        sig_p = P("sig", 2)
        cols_p = P("cols", 2)
        feat_p = P("feat", 1)     # K1T/Q1T/Ktd per chunk (all heads)
        ftmp_p = P("ftmp", B("FTB", 6))     # xp, f fp32 temps
        sc_p = P("sc", B("SCB", 8))         # scan sbuf temps (N', B', ...)
        bd_p = P("bd", B("BDB", 8))         # bf16 [128,128] scratch for stt outs
        outT_p = P("outT", 2)
        xln_p = P("xln", 2)
        # PSUM budget: 8 banks total -> blk(3) + tp(2) + s(2) + pAT(1)
        psA_p = P("psA", B("BLKB", 3), space="PSUM")      # proj/A/S1/S2 rotating (tag blk)
        psT_p = P("psT", B("TPB", 2), space="PSUM")      # transpose outs (tag tp)
        psS_p = P("psS", B("SB", 2), space="PSUM")      # solve/state (tag s)
        psO_p = P("psO", 1, space="PSUM")      # out-proj (tag pAT)

        # ---- constants
        t_mSL = const.tile([128, 132], F32, tag="mSL", name="mSL"); nc.sync.dma_start(t_mSL[:], d_mSL[:])
        t_mLI = const.tile([128, 132], F32, tag="mLI", name="mLI"); nc.sync.dma_start(t_mLI[:], d_mLI[:])
        t_mUI = const.tile([128, 132], F32, tag="mUI", name="mUI"); nc.sync.dma_start(t_mUI[:], d_mUI[:])
        t_id = const.tile([128, 128], BF16, tag="id", name="id"); nc.sync.dma_start(t_id[:], d_id[:])
        t_lng = const.tile([128, DM], F32, tag="lng", name="lng"); nc.sync.dma_start(t_lng[:], d_lng[:])
        t_lnb = const.tile([128, DM], F32, tag="lnb", name="lnb"); nc.sync.dma_start(t_lnb[:], d_lnb[:])
        t_w = []
        for mc in range(8):
            t = const.tile([128, OTOT], BF16, tag=f"w{mc}", name=f"w{mc}")
            nc.sync.dma_start(t[:], d_w[mc * 128:(mc + 1) * 128, :])
            t_w.append(t)
        t_wo = []
        for ic in range(8):
            t = const.tile([128, DM], BF16, tag=f"wo{ic}", name=f"wo{ic}")
            nc.sync.dma_start(t[:], d_wo[ic * 128:(ic + 1) * 128, :])
            t_wo.append(t)

        # ---- state: per-head W ([128, 128]: dc0 cols 0:64, dc1 64:128), r
        t_Wm, t_Wb = [], []
        for hd in range(n_heads):
            wm = state.tile([128, 128], F32, tag=f"wm{hd}", name=f"wm{hd}")
            nc.vector.memset(wm[:], 0.0)
            wb = state.tile([128, 128], BF16, tag=f"wb{hd}", name=f"wb{hd}")
            nc.vector.memset(wb[:], 0.0)
            t_Wm.append(wm); t_Wb.append(wb)
        t_r = []
        for dc in range(2):
            r = state.tile([128, NH], F32, tag=f"r{dc}", name=f"r{dc}")
            nc.vector.memset(r[:], 0.0)
            t_r.append(r)

        for c in range(n_chunks):
            cs = slice(c * 128, (c + 1) * 128)
            # ================= projection: out[t, o] for this chunk =========
            hts = hts_p.tile([128, 1024], BF16, tag="hts", name="hts")
            for mc in range(8):
                nc.sync.dma_start(hts[:, mc * 128:(mc + 1) * 128],
                                  d_hT[mc * 128:(mc + 1) * 128, cs])
            raw = raw_p.tile([128, OTOT], BF16, tag="raw", name="raw")
            ogs = [(i * 512, 512) for i in range(6)] + [(OQKV, NH)]
            for (o0, ow) in ogs:
                pg = psA_p.tile([128, ow], F32, tag="blk", name="blk")
                for mc in range(8):
                    nc.tensor.matmul(pg[:], hts[:, mc * 128:(mc + 1) * 128],
                                     t_w[mc][:, o0:o0 + ow],
                                     start=(mc == 0), stop=(mc == 7))
                nc.vector.tensor_copy(raw[:, o0:o0 + ow], pg[:])
            sig = sig_p.tile([128, NH], F32, tag="sig", name="sig")
            nc.scalar.activation(sig[:], raw[:, OQKV:OQKV + NH], AF.Sigmoid)

            # ================= features (all heads) =========================
            K1T, Q1T, Ktd = [], [], []
            for hd in range(n_heads):
                k1t = [feat_p.tile([128, 132], BF16, tag=f"k1t{hd}_{dc}", name=f"k1t{hd}_{dc}")
                       for dc in range(2)]
                q1t = [feat_p.tile([128, 128], BF16, tag=f"q1t{hd}_{dc}", name=f"q1t{hd}_{dc}")
                       for dc in range(2)]
                ktd = feat_p.tile([128, 256], BF16, tag=f"ktd{hd}", name=f"ktd{hd}")
                K1T.append(k1t); Q1T.append(q1t); Ktd.append(ktd)
                qoff = hd * 192
                for (src_off, is_k) in ((qoff, 0), (qoff + 64, 1)):
                    xp = ftmp_p.tile([128, 128], F32, tag="xp", name="xp")
                    nc.scalar.activation(xp[:, 0:64], raw[:, src_off:src_off + 64],
                                         AF.Relu)
                    nc.scalar.activation(xp[:, 64:128], raw[:, src_off:src_off + 64],
                                         AF.Relu, scale=-1.0)
                    f = ftmp_p.tile([128, 256], F32, tag="f", name="f")
                    nc.vector.tensor_mul(f[:, 1:128], xp[:, 1:128], xp[:, 0:127])
                    nc.vector.tensor_mul(f[:, 0:1], xp[:, 0:1], xp[:, 127:128])
                    nc.vector.tensor_mul(f[:, 130:256], xp[:, 2:128], xp[:, 0:126])
                    nc.vector.tensor_mul(f[:, 128:130], xp[:, 0:2], xp[:, 126:128])
                    fsum = ftmp_p.tile([128, 1], F32, tag="fsum", name="fsum")
                    nc.vector.tensor_reduce(fsum[:], f[:], AX.X, OP.add)
                    frec = ftmp_p.tile([128, 1], F32, tag="frec", name="frec")
                    nc.vector.reciprocal(frec[:], fsum[:])
                    if is_k:
                        td = ktd
                    else:
                        td = ftmp_p.tile([128, 256], BF16, tag="qtd", name="qtd")
                    nc.scalar.mul(td[:], f[:], frec[:])
                    dst = k1t if is_k else q1t
                    for dc in range(2):
                        pt = psT_p.tile([128, 128], BF16, tag="tp", name="tp")
                        nc.tensor.transpose(pt[:], td[:, dc * 128:(dc + 1) * 128],
                                            t_id[:])
                        nc.vector.tensor_copy(dst[dc][:, 0:128], pt[:])
                for dc in range(2):
                    nc.gpsimd.tensor_copy(k1t[dc][:, 128:129],
                                          t_r[dc][:, hd:hd + 1])

            # ================= scan pass 1: block-diag matmuls ==============
            kd_all = cols_p.tile([128, NH], F32, tag="kd", name="kd")
            dn_all = cols_p.tile([128, NH], F32, tag="dn", name="dn")
            Shat, Araw = [], []
            for hd in range(n_heads):
                pA = psA_p.tile([128, 132], F32, tag="blk", name="blk")
                pS1 = psA_p.tile([128, 128], F32, tag="blk", name="blk")
                pS2 = psA_p.tile([128, 132], F32, tag="blk", name="blk")
                for dc in range(2):
                    st = (dc == 0); sp = (dc == 1)
                    nc.tensor.matmul(pA[:, 0:129], K1T[hd][dc][:, 0:128],
                                     K1T[hd][dc][:, 0:129], start=st, stop=sp)
                    nc.tensor.matmul(pS1[:], K1T[hd][dc][:, 0:128],
                                     Q1T[hd][dc][:], start=st, stop=sp)
                    nc.tensor.matmul(pS2[:, 0:129], Q1T[hd][dc][:],
                                     K1T[hd][dc][:, 0:129], start=st, stop=sp)
                scr = bd_p.tile([128, 132], BF16, tag="scr", name="scr")
                nc.vector.scalar_tensor_tensor(scr[:, 0:129], pA[:, 0:129], 1.0,
                                               t_mSL[:, 0:129], OP.mult, OP.mult,
                                               accum_out=kd_all[:, hd:hd + 1])
                if c == 0:
                    nc.vector.memset(kd_all[0:1, hd:hd + 1], 1.0)
                scr2 = bd_p.tile([128, 132], BF16, tag="scr", name="scr")
                nc.vector.scalar_tensor_tensor(scr2[:, 0:129], pS2[:, 0:129], 1.0,
                                               t_mLI[:, 0:129], OP.mult, OP.mult,
                                               accum_out=dn_all[:, hd:hd + 1])
                sh = sc_p.tile([128, 128], BF16, tag=f"sh{hd % 4}", name=f"sh{hd % 4}")
                nc.vector.tensor_mul(sh[:], pS1[:], t_mUI[:, 0:128])
                ar = sc_p.tile([128, 128], BF16, tag=f"ar{hd % 4}", name=f"ar{hd % 4}")
                nc.vector.tensor_copy(ar[:], pA[:, 0:128])
                Shat.append(sh); Araw.append(ar)

            # ================= chunk-level columns ==========================
            ceps = cols_p.tile([128, NH], F32, tag="ceps", name="ceps")
            nc.vector.tensor_scalar_add(ceps[:], kd_all[:], EPS)
            c_all = cols_p.tile([128, NH], F32, tag="c", name="c")
            nc.vector.reciprocal(c_all[:], ceps[:])
            cneg = cols_p.tile([128, NH], F32, tag="cneg", name="cneg")
            nc.vector.tensor_scalar_mul(cneg[:], c_all[:], -1.0)
            t0 = cols_p.tile([128, NH], F32, tag="t0", name="t0")
            nc.vector.tensor_mul(t0[:], kd_all[:], c_all[:])
            cb_all = cols_p.tile([128, NH], F32, tag="cb", name="cb")
            nc.vector.tensor_mul(cb_all[:], t0[:], sig[:])
            cbc = cols_p.tile([128, NH], F32, tag="cbc", name="cbc")
            nc.vector.tensor_mul(cbc[:], cb_all[:], c_all[:])
            dne = cols_p.tile([128, NH], F32, tag="dne", name="dne")
            nc.vector.tensor_scalar_add(dne[:], dn_all[:], EPS)
            dnr = cols_p.tile([128, NH], F32, tag="dnr", name="dnr")
            nc.vector.reciprocal(dnr[:], dne[:])
            dnrS = cols_p.tile([128, NH], F32, tag="dnrS", name="dnrS")
            nc.vector.tensor_scalar_mul(dnrS[:], dnr[:], SCALE)

            # ================= scan pass 2: solve + outputs + W =============
            outT = [outT_p.tile([128, 128], BF16, tag=f"oT{ic}", name=f"oT{ic}")
                    for ic in range(n_heads // 2)]
            for hd in range(n_heads):
                voff = hd * 192 + 128
                # N'' and B'' = N''^T
                Nt = sc_p.tile([128, 128], BF16, tag="Nt", name="Nt")
                nc.vector.scalar_tensor_tensor(Nt[:], Araw[hd][:], cbc[:, hd:hd + 1],
                                               t_mSL[:, 0:128], OP.mult, OP.mult)
                pB = psT_p.tile([128, 128], BF16, tag="tp", name="tp")
                nc.tensor.transpose(pB[:], Nt[:], t_id[:])
                Bt = sc_p.tile([128, 128], BF16, tag="Bt", name="Bt")
                nc.vector.tensor_copy(Bt[:], pB[:])
                pB2 = psS_p.tile([128, 128], F32, tag="s", name="s")
                nc.tensor.matmul(pB2[:], Nt[:], Bt[:], start=True, stop=True)
                B2t = sc_p.tile([128, 128], BF16, tag="B2t", name="B2t")
                nc.vector.tensor_copy(B2t[:], pB2[:])
                # X0 = cb*(V - c*KW)
                X0 = sc_p.tile([128, 64], BF16, tag="X0", name="X0")
                if c == 0:
                    nc.scalar.mul(X0[:], raw[:, voff:voff + 64], cb_all[:, hd:hd + 1])
                else:
                    pKW = psS_p.tile([128, 64], F32, tag="s", name="s")
                    for dc in range(2):
                        nc.tensor.matmul(pKW[:], K1T[hd][dc][:, 0:128],
                                         t_Wb[hd][:, dc * 64:(dc + 1) * 64],
                                         start=(dc == 0), stop=(dc == 1))
                    x0f = ftmp_p.tile([128, 64], F32, tag="x0f", name="x0f")
                    nc.vector.scalar_tensor_tensor(x0f[:], pKW[:], cneg[:, hd:hd + 1],
                                                   raw[:, voff:voff + 64],
                                                   OP.mult, OP.add)
                    nc.scalar.mul(X0[:], x0f[:], cb_all[:, hd:hd + 1])
                # X1 = X0 + N''^2 X0 ; Y = X1 - N'' X1
                pX = psS_p.tile([128, 64], F32, tag="s", name="s")
                nc.tensor.matmul(pX[:], B2t[:], X0[:], start=True, stop=True)
                X1 = sc_p.tile([128, 64], BF16, tag="X1", name="X1")
                nc.vector.tensor_add(X1[:], pX[:], X0[:])
                pY = psS_p.tile([128, 64], F32, tag="s", name="s")
                nc.tensor.matmul(pY[:], Bt[:], X1[:], start=True, stop=True)
                Yt = sc_p.tile([128, 64], BF16, tag="Yt", name="Yt")
                nc.vector.tensor_sub(Yt[:], X1[:], pY[:])
                # Out = QW + tril(S1) Y, scaled by SCALE/(denom+eps)
                pOut = psS_p.tile([128, 64], F32, tag="s", name="s")
                if c == 0:
                    nc.tensor.matmul(pOut[:], Shat[hd][:], Yt[:],
                                     start=True, stop=True)
                else:
                    for dc in range(2):
                        nc.tensor.matmul(pOut[:], Q1T[hd][dc][:],
                                         t_Wb[hd][:, dc * 64:(dc + 1) * 64],
                                         start=(dc == 0), stop=False)
                    nc.tensor.matmul(pOut[:], Shat[hd][:], Yt[:],
                                     start=False, stop=True)
                outc = sc_p.tile([128, 64], BF16, tag="outc", name="outc")
                nc.scalar.mul(outc[:], pOut[:], dnrS[:, hd:hd + 1])
                pT = psT_p.tile([128, 128], BF16, tag="tp", name="tp")
                base = (hd % 2) * 64
                nc.tensor.transpose(pT[base:base + 64, :], outc[:], t_id[:],
                                    tile_position=(0, base))
                nc.vector.tensor_copy(outT[hd // 2][base:base + 64, :],
                                      pT[base:base + 64, :])
                # W update: W += K1^T Y  (via Ktd), r update
                pW = psS_p.tile([128, 128], F32, tag="s", name="s")
                for dc in range(2):
                    nc.tensor.matmul(pW[:, dc * 64:(dc + 1) * 64],
                                     Ktd[hd][:, dc * 128:(dc + 1) * 128], Yt[:],
                                     start=True, stop=True)
                nc.vector.tensor_add(t_Wm[hd][:], pW[:], t_Wm[hd][:])
                nc.gpsimd.tensor_copy(t_Wb[hd][:], t_Wm[hd][:])
                for dc in range(2):
                    rs = ftmp_p.tile([128, 1], F32, tag="rs", name="rs")
                    nc.vector.tensor_reduce(rs[:], K1T[hd][dc][:, 0:128], AX.X,
                                            OP.add)
                    nc.vector.tensor_add(t_r[dc][:, hd:hd + 1],
                                         t_r[dc][:, hd:hd + 1], rs[:])

            # ================= output projection + residual + LN ============
            hr = xln_p.tile([128, DM], F32, tag="hr", name="hr")
            nc.sync.dma_start(hr[:], d_hres[cs, :])
            x = xln_p.tile([128, DM], F32, tag="x", name="x")
            for og in range(2):
                pAT = psO_p.tile([128, 512], F32, tag="pAT", name="pAT")
                for ic in range(n_heads // 2):
                    nc.tensor.matmul(pAT[:], outT[ic][:],
                                     t_wo[ic][:, og * 512:(og + 1) * 512],
                                     start=(ic == 0), stop=(ic == n_heads // 2 - 1))
                nc.vector.tensor_add(x[:, og * 512:(og + 1) * 512], pAT[:],
                                     hr[:, og * 512:(og + 1) * 512])
            xsum = ftmp_p.tile([128, 1], F32, tag="xsum", name="xsum")
            nc.vector.tensor_reduce(xsum[:], x[:], AX.X, OP.add)
            nmu = ftmp_p.tile([128, 1], F32, tag="nmu", name="nmu")
            nc.vector.tensor_scalar_mul(nmu[:], xsum[:], -1.0 / DM)
            nc.vector.tensor_scalar_add(x[:], x[:], nmu[:])
            vscr = xln_p.tile([128, DM], BF16, tag="vscr", name="vscr")
            var = ftmp_p.tile([128, 1], F32, tag="var", name="var")
            nc.vector.scalar_tensor_tensor(vscr[:], x[:], 1.0, x[:],
                                           OP.mult, OP.mult, accum_out=var[:])
            vare = ftmp_p.tile([128, 1], F32, tag="vare", name="vare")
            nc.vector.tensor_scalar(vare[:], var[:], 1.0 / DM, float(LN_EPS),
                                    OP.mult, OP.add)
            sd = ftmp_p.tile([128, 1], F32, tag="sd", name="sd")
            nc.scalar.sqrt(sd[:], vare[:])
            rstd = ftmp_p.tile([128, 1], F32, tag="rstd", name="rstd")
            nc.vector.reciprocal(rstd[:], sd[:])
            nc.vector.scalar_tensor_tensor(x[:], x[:], rstd[:], t_lng[:],
                                           OP.mult, OP.mult)
            nc.vector.tensor_add(x[:], x[:], t_lnb[:])
            nc.sync.dma_start(d_out[cs, :], x[:])

    return nc


# ---------------------------------------------------------------- host side
def _prep_core_inputs(h_b, W_qkvb, W_o, ln_g, ln_b):
    bf16 = ml_dtypes.bfloat16
    hT = np.ascontiguousarray(h_b.T).astype(bf16)                  # [1024, 2048]
    wq = np.zeros((DM, OTOT), dtype=bf16)
    Wr = W_qkvb.reshape(NH, 193, DM)
    for hd in range(NH):
        wq[:, hd * 192:hd * 192 + 192] = Wr[hd, 0:192, :].T
        wq[:, OQKV + hd] = Wr[hd, 192, :]
    woT = np.ascontiguousarray(W_o.T).astype(bf16)                 # [i, o]
    lng = np.broadcast_to(ln_g[None, :], (128, DM)).astype(np.float32).copy()
    lnb = np.broadcast_to(ln_b[None, :], (128, DM)).astype(np.float32).copy()
    ii, jj = np.indices((128, 132))
    mSL = (jj < ii).astype(np.float32);  mSL[:, 128] = 1.0
    mLI = (jj <= ii).astype(np.float32); mLI[:, 128] = 1.0
    mUI = ((jj >= ii) & (jj < 128)).astype(np.float32)
    identb = np.eye(128, dtype=bf16)
    return {"hT": hT, "hres": np.ascontiguousarray(h_b, np.float32),
            "wqkv": wq, "woT": woT, "lng": lng, "lnb": lnb,
            "maskSL": mSL, "maskLI": mLI, "maskUI": mUI, "identb": identb}


_cached = {}


def kernel(h, W_qkvb, W_o, ln_g, ln_b):
    h = np.asarray(h, np.float32)
    W_qkvb = np.asarray(W_qkvb, np.float32)
    W_o = np.asarray(W_o, np.float32)
    ln_g = np.asarray(ln_g, np.float32)
    ln_b = np.asarray(ln_b, np.float32)
    if "nc" not in _cached:
        _cached["nc"] = build_program()
    nc = _cached["nc"]
    in_maps = [_prep_core_inputs(h[:, b, :], W_qkvb, W_o, ln_g, ln_b)
               for b in range(BSZ)]
    res = run_bass_kernel_spmd(nc, in_maps, list(range(BSZ)),
                               trace=os.environ.get("BASS_TRACE", "") == "1")
    out = np.stack([res.results[b]["out"] for b in range(BSZ)], axis=1)
    kernel.last_exec_time_ns = res.exec_time_ns
    return out.astype(np.float32)



# revision 4
# speedup vs baseline: 74.5058x; 74.5058x over previous
"""Trainium2 Bass kernel for the CudaNorm FastWeight DPFP transformer layer.

Sharding: batch (8) across the 8 cores; each core runs its batch's full layer.

Redesign vs v1: head-batched DVE/Act ops via 3D strided APs, [K|Q|V|b]
projection layout, fused per-head gram matmuls ([S1|A|K.r] one psum group),
unnormalized-Q trick (fold 1/sum_Q into dn/dnrS), Horner-form depth-2 Neumann
solve with batched cbc post-scaling, stage-major scan with 8-head psum
packing, and prepA/scan/prepB software pipelining across chunks.
"""
import os
import numpy as np
import ml_dtypes

import concourse.bass as bass
import concourse.mybir as mybir
from concourse.bass_utils import run_bass_kernel_spmd
from concourse.tile import TileContext
from concourse.vector_clock import ScopedClock, VectorClock
from contextlib import ExitStack

F32 = mybir.dt.float32
BF16 = mybir.dt.bfloat16
AF = mybir.ActivationFunctionType
OP = mybir.AluOpType
AX = mybir.AxisListType

SLEN, BSZ, DM = 2048, 8, 1024
NH, DH, NROLL = 16, 64, 2
D = 2 * NROLL * DH            # 256 feature dim
C = 128                       # chunk length
NCH = SLEN // C               # 16 chunks
EPS, LN_EPS = 1e-5, 1e-5
SCALE = 1.0 / float(np.sqrt(DH))
GW = 260                      # per-head stride in G tiles: [Q 128 | K 128 | r 1 | pad]
NHORN = 2                     # Horner/Neumann solve iterations

# ---------------------------------------------------------------- tile ctx
MAXW = 2


class PatchedTileContext(TileContext):
    """Work around walrus TPB sync-command limits: each instruction carries at
    most 2 sync commands (waits+updates); hoist excess waits onto preceding
    same-engine NoOps (1 wait each), and emit the kernel-tail drain's waits
    one-per-nop on SP."""

    def _lower_ordered_insts(self, ordered):
        for bb_name in list(ordered.keys()):
            new = []
            for inst in ordered[bb_name]:
                si = inst.sync_info
                nupd = len(si.on_update) if si is not None and si.on_update else 0
                maxw = max(0, MAXW - nupd)
                if si is not None and si.on_wait and len(si.on_wait) > maxw:
                    waits = list(si.on_wait)
                    excess = waits if maxw == 0 else waits[:-maxw]
                    keep = [] if maxw == 0 else waits[-maxw:]
                    for w in excess:
                        nop = mybir.InstNoOp(
                            name=self.nc.get_next_instruction_name(),
                            engine=inst.engine, ins=[], outs=[])
                        nop.sync_info = mybir.SyncInfo(on_wait=[w], on_update=[])
                        new.append(nop)
                    inst.sync_info = mybir.SyncInfo(
                        on_wait=keep, on_update=list(si.on_update or []))
                new.append(inst)
            ordered[bb_name] = new
        return super()._lower_ordered_insts(ordered)

    def _drain_and_barrier(self, tick_clock, wait_clock):
        gc = tick_clock.global_clock
        n = len(gc)
        for p in range(n):
            if gc[p] > 0:
                vc = VectorClock([gc[i] if i == p else 0 for i in range(n)])
                nop = self.nc.sync.nop(nofuse=True)
                wait_clock.add_sem_waits(nop.ins, ScopedClock({None: vc}))
        self.nc.sync.drain()
        self.nc.all_engine_barrier()
        assert self.sems is not None
        popped = self.nc._tile_sem_poison_stack.pop()
        assert popped is self._sem_poison
        self.nc.clear_and_free_semaphores(list(self.sems.allocated().values()))
        self.nc.all_engine_barrier()


# ---------------------------------------------------------------- program
def build_program(n_chunks=NCH):
    nc = bass.Bass()
    d_hT = nc.declare_dram_parameter("hT", [DM, SLEN], BF16, isOutput=False)
    d_hres = nc.declare_dram_parameter("hres", [SLEN, DM], BF16, isOutput=False)
    d_w = nc.declare_dram_parameter("wqkv", [DM, 3088], BF16, isOutput=False)
    d_wo = nc.declare_dram_parameter("woT", [DM, DM], BF16, isOutput=False)
    d_lng = nc.declare_dram_parameter("lng", [128, DM], BF16, isOutput=False)
    d_lnb = nc.declare_dram_parameter("lnb", [128, DM], BF16, isOutput=False)
    d_mSL = nc.declare_dram_parameter("maskSL", [128, 132], BF16, isOutput=False)
    d_mLI = nc.declare_dram_parameter("maskLI", [128, 132], BF16, isOutput=False)
    d_mUI = nc.declare_dram_parameter("maskUI", [128, 128], BF16, isOutput=False)
    d_mUS = nc.declare_dram_parameter("maskUS", [128, 128], BF16, isOutput=False)
    d_id = nc.declare_dram_parameter("identb", [128, 128], BF16, isOutput=False)
    d_out = nc.declare_dram_parameter("out", [SLEN, DM], F32, isOutput=True)

    with PatchedTileContext(nc) as tc, ExitStack() as ctx:
        P = lambda name, bufs, **kw: ctx.enter_context(
            tc.tile_pool(name=name, bufs=bufs, **kw))
        const = P("const", 1)
        state = P("state", 1)
        hts_p = P("hts", 2)
        xp_p = P("xp", 1)
        f_p = P("f", 1)
        tdK_p = P("tdK", 2)
        G_p = P("G", 2)
        V_p = P("V", 1)
        t1_p = P("t1", 2)
        cols_p = P("cols", 2)
        scr_p = P("scr", 4)
        sa_p = P("sa", 3)
        sh_p = P("sh", 2)
        nt_p = P("nt", 2)
        x0_p = P("x0", 1)
        z_p = P("z", 1)
        mg_p = P("mg", 3)
        oc_p = P("oc", 1)
        oT_p = P("oT", 1)
        hr_p = P("hr", 2)
        x_p = P("x", 2)
        ft_p = P("ft", 2)
        psP_p = P("psP", 2, space="PSUM")
        psT_p = P("psT", 2, space="PSUM")
        psG_p = P("psG", 2, space="PSUM")
        psS_p = P("psS", 2, space="PSUM")

        # ---- constants
        t_mSL = const.tile([128, 132], BF16, tag="mSL", name="mSL"); nc.sync.dma_start(t_mSL[:], d_mSL[:])
        t_mLI = const.tile([128, 132], BF16, tag="mLI", name="mLI"); nc.sync.dma_start(t_mLI[:], d_mLI[:])
        t_mUI = const.tile([128, 128], BF16, tag="mUI", name="mUI"); nc.sync.dma_start(t_mUI[:], d_mUI[:])
        t_mUS = const.tile([128, 128], BF16, tag="mUS", name="mUS"); nc.sync.dma_start(t_mUS[:], d_mUS[:])
        t_id = const.tile([128, 128], BF16, tag="id", name="id"); nc.sync.dma_start(t_id[:], d_id[:])
        t_lng = const.tile([128, DM], BF16, tag="lng", name="lng"); nc.sync.dma_start(t_lng[:], d_lng[:])
        t_lnb = const.tile([128, DM], BF16, tag="lnb", name="lnb"); nc.sync.dma_start(t_lnb[:], d_lnb[:])
        t_w = []
        for mc in range(8):
            t = const.tile([128, 3088], BF16, tag=f"w{mc}", name=f"w{mc}")
            nc.sync.dma_start(t[:], d_w[mc * 128:(mc + 1) * 128, :])
            t_w.append(t)
        t_wo = []
        for ic in range(8):
            t = const.tile([128, DM], BF16, tag=f"wo{ic}", name=f"wo{ic}")
            nc.sync.dma_start(t[:], d_wo[ic * 128:(ic + 1) * 128, :])
            t_wo.append(t)

        # ---- state
        t_Wm = state.tile([128, NH * 128], F32, tag="Wm", name="Wm")
        nc.vector.memset(t_Wm[:], 0.0)
        t_Wb = state.tile([128, NH * 128], BF16, tag="Wb", name="Wb")
        nc.vector.memset(t_Wb[:], 0.0)
        t_r = []
        for dc in range(2):
            r = state.tile([128, NH], F32, tag=f"r{dc}", name=f"r{dc}")
            nc.vector.memset(r[:], 0.0)
            t_r.append(r)

        # per-chunk tile handles passed prepA -> prepB -> scan
        H = {}

        def prepA(c):
            cs = slice(c * 128, (c + 1) * 128)
            hts = hts_p.tile([128, 1024], BF16, tag="hts", name="hts")
            nc.sync.dma_start(
                hts[:].rearrange("p (mc t) -> p mc t", t=128),
                d_hT[:, cs].rearrange("(mc p) t -> p mc t", p=128))
            hr = hr_p.tile([128, DM], BF16, tag="hr", name="hr")
            nc.sync.dma_start(hr[:], d_hres[cs, :])
            xpK = xp_p.tile([128, 2048], BF16, tag="xpK", name="xpK")
            xpQ = xp_p.tile([128, 2048], BF16, tag="xpQ", name="xpQ")
            Vall = V_p.tile([128, 1024], BF16, tag="V", name="V")
            sig = cols_p.tile([128, NH], F32, tag="sig", name="sig")
            for og in range(6):
                ps = psP_p.tile([128, 512], F32, tag="pp", name="pp")
                for mc in range(8):
                    nc.tensor.matmul(ps[:], hts[:, mc * 128:(mc + 1) * 128],
                                     t_w[mc][:, og * 512:(og + 1) * 512],
                                     start=(mc == 0), stop=(mc == 7))
                psr = ps[:].rearrange("p (h v) -> p h v", v=64)
                side, g = divmod(og, 2) if og < 4 else (None, None)
                if og < 2:      # K heads 8*og..8*og+7
                    xr = xpK[:].rearrange("p (h w) -> p h w", w=128)
                    nc.scalar.activation(xr[:, og * 8:(og + 1) * 8, 0:64], psr, AF.Relu)
                    nc.scalar.activation(xr[:, og * 8:(og + 1) * 8, 64:128], psr,
                                         AF.Relu, scale=-1.0)
                elif og < 4:    # Q heads
                    xr = xpQ[:].rearrange("p (h w) -> p h w", w=128)
                    hq = og - 2
                    nc.scalar.activation(xr[:, hq * 8:(hq + 1) * 8, 0:64], psr, AF.Relu)
                    nc.scalar.activation(xr[:, hq * 8:(hq + 1) * 8, 64:128], psr,
                                         AF.Relu, scale=-1.0)
                else:           # V
                    gv = og - 4
                    nc.scalar.copy(Vall[:, gv * 512:(gv + 1) * 512], ps[:])
            psb = psP_p.tile([128, NH], F32, tag="pp", name="pp")
            for mc in range(8):
                nc.tensor.matmul(psb[:], hts[:, mc * 128:(mc + 1) * 128],
                                 t_w[mc][:, 3072:3088],
                                 start=(mc == 0), stop=(mc == 7))
            nc.scalar.activation(sig[:], psb[:], AF.Sigmoid)
            H[c] = dict(xpK=xpK, xpQ=xpQ, Vall=Vall, sig=sig, hr=hr)

        def feats(xp, side):
            """xp [128, 16*128] -> f [128, 16*256] roll-products + fp32 sums."""
            f = f_p.tile([128, 4096], BF16, tag="f", name="f")
            fr = f[:].rearrange("p (h w) -> p h w", w=256)
            xr = xp[:].rearrange("p (h w) -> p h w", w=128)
            nc.vector.tensor_mul(fr[:, :, 1:128], xr[:, :, 1:128], xr[:, :, 0:127])
            nc.vector.tensor_mul(fr[:, :, 0:1], xr[:, :, 0:1], xr[:, :, 127:128])
            nc.vector.tensor_mul(fr[:, :, 130:256], xr[:, :, 2:128], xr[:, :, 0:126])
            nc.vector.tensor_mul(fr[:, :, 128:130], xr[:, :, 0:2], xr[:, :, 126:128])
            fsum = cols_p.tile([128, NH], F32, tag=f"fsum{side}", name=f"fsum{side}")
            nc.vector.tensor_reduce(fsum[:], fr, AX.X, OP.add)
            frec = cols_p.tile([128, NH], F32, tag=f"frec{side}", name=f"frec{side}")
            nc.vector.reciprocal(frec[:], fsum[:])
            return f, frec

        def prepB(c):
            h = H[c]
            # K features (normalized into tdK), Q features (unnormalized f)
            fK, frecK = feats(h["xpK"], "K")
            tdK = tdK_p.tile([128, 4096], BF16, tag="tdK", name="tdK")
            nc.vector.tensor_tensor(
                tdK[:].rearrange("p (h w) -> p h w", w=256),
                fK[:].rearrange("p (h w) -> p h w", w=256),
                frecK[:].unsqueeze(2).broadcast_to((128, NH, 256)), OP.mult)
            fQ, frecQ = feats(h["xpQ"], "Q")

            # G tiles: [Q 128 | K 128 | r 1] per head, per dc
            G = [G_p.tile([128, NH * GW], BF16, tag=f"G{dc}", name=f"G{dc}")
                 for dc in range(2)]
            Gr = [G[dc][:].rearrange("p (h w) -> p h w", w=GW) for dc in range(2)]
            for dc in range(2):
                nc.gpsimd.tensor_copy(Gr[dc][:, :, 256:257], t_r[dc][:].unsqueeze(2))
            # transposes: 4 groups (side, dc) x 2 halves; 8 per psum bank
            for (src, colbase) in ((tdK, 128), (fQ, 0)):
                for dc in range(2):
                    for half in range(2):
                        pt = psT_p.tile([128, 1024], BF16, tag="tp", name="tp")
                        for j in range(8):
                            hd = half * 8 + j
                            nc.tensor.transpose(
                                pt[:, j * 128:(j + 1) * 128],
                                src[:, hd * 256 + dc * 128: hd * 256 + dc * 128 + 128],
                                t_id[:])
                        nc.scalar.copy(
                            Gr[dc][:, half * 8:(half + 1) * 8,
                                   colbase:colbase + 128],
                            pt[:].rearrange("p (h w) -> p h w", w=128))
            # r state update (prep-side): r += per-head column sums of K1T
            for dc in range(2):
                rs = cols_p.tile([128, NH], F32, tag=f"rs{dc}", name=f"rs{dc}")
                nc.vector.tensor_reduce(rs[:], Gr[dc][:, :, 128:256], AX.X, OP.add)
                nc.vector.tensor_add(t_r[dc][:], rs[:], t_r[dc][:])

            # grams + masked reductions per head
            kd = cols_p.tile([128, NH], F32, tag="kd", name="kd")
            dna = cols_p.tile([128, NH], F32, tag="dna", name="dna")
            Shat = sh_p.tile([128, NH * 128], BF16, tag="sh", name="sh")
            NT = nt_p.tile([128, NH * 128], BF16, tag="nt", name="nt")
            for hd in range(NH):
                ps = psG_p.tile([128, 386], F32, tag="gg", name="gg")
                b = hd * GW
                for dc in range(2):
                    st = (dc == 0); sp = (dc == 1)
                    nc.tensor.matmul(ps[:, 0:257], G[dc][:, b + 128:b + 256],
                                     G[dc][:, b:b + 257], start=st, stop=sp)
                    nc.tensor.matmul(ps[:, 257:386], G[dc][:, b:b + 128],
                                     G[dc][:, b + 128:b + 257], start=st, stop=sp)
                scr = scr_p.tile([128, 132], BF16, tag="scr", name="scr")
                nc.vector.scalar_tensor_tensor(scr[:, 0:129], ps[:, 128:257], 1.0,
                                               t_mSL[:, 0:129], OP.mult, OP.mult,
                                               accum_out=kd[:, hd:hd + 1])
                scr2 = scr_p.tile([128, 132], BF16, tag="scr", name="scr")
                nc.vector.scalar_tensor_tensor(scr2[:, 0:129], ps[:, 257:386], 1.0,
                                               t_mLI[:, 0:129], OP.mult, OP.mult,
                                               accum_out=dna[:, hd:hd + 1])
                sa = sa_p.tile([128, 256], BF16, tag="sa", name="sa")
                nc.vector.tensor_copy(sa[:], ps[:, 0:256])
                nc.gpsimd.tensor_mul(Shat[:, hd * 128:(hd + 1) * 128],
                                     sa[:, 0:128], t_mUI[:])
                nc.gpsimd.tensor_mul(NT[:, hd * 128:(hd + 1) * 128],
                                     sa[:, 128:256], t_mUS[:])
            if c == 0:
                nc.vector.memset(kd[0:1, :], 1.0)

            # chunk-level columns
            ceps = cols_p.tile([128, NH], F32, tag="ceps", name="ceps")
            nc.vector.tensor_scalar_add(ceps[:], kd[:], EPS)
            cc = cols_p.tile([128, NH], F32, tag="cc", name="cc")
            nc.vector.reciprocal(cc[:], ceps[:])
            t0 = cols_p.tile([128, NH], F32, tag="t0", name="t0")
            nc.vector.tensor_mul(t0[:], kd[:], cc[:])
            cb = cols_p.tile([128, NH], F32, tag="cb", name="cb")
            nc.vector.tensor_mul(cb[:], t0[:], h["sig"][:])
            cbc = cols_p.tile([128, NH], F32, tag="cbc", name="cbc")
            nc.vector.tensor_mul(cbc[:], cb[:], cc[:])
            dnm = cols_p.tile([128, NH], F32, tag="dnm", name="dnm")
            nc.vector.tensor_mul(dnm[:], dna[:], frecQ[:])
            dne = cols_p.tile([128, NH], F32, tag="dne", name="dne")
            nc.vector.tensor_scalar_add(dne[:], dnm[:], EPS)
            dnr = cols_p.tile([128, NH], F32, tag="dnr", name="dnr")
            nc.vector.reciprocal(dnr[:], dne[:])
            dnr2 = cols_p.tile([128, NH], F32, tag="dnr2", name="dnr2")
            nc.vector.tensor_scalar_mul(dnr2[:], dnr[:], SCALE)
            dnrS = cols_p.tile([128, NH], F32, tag="dnrS", name="dnrS")
            nc.vector.tensor_mul(dnrS[:], dnr2[:], frecQ[:])
            # t1 = cb * V
            t1 = t1_p.tile([128, 1024], BF16, tag="t1", name="t1")
            nc.vector.tensor_tensor(
                t1[:].rearrange("p (h v) -> p h v", v=64),
                h["Vall"][:].rearrange("p (h v) -> p h v", v=64),
                cb[:].unsqueeze(2).broadcast_to((128, NH, 64)), OP.mult)
            h.update(G=G, tdK=tdK, Shat=Shat, NT=NT, cbc=cbc, cc=cc,
                     dnrS=dnrS, t1=t1)

        def bscale(ps, colvec, g):
            """bf16 tile = psum [128,512] * per-head column broadcast."""
            mg = mg_p.tile([128, 512], BF16, tag="mg", name="mg")
            nc.vector.tensor_tensor(
                mg[:].rearrange("p (h v) -> p h v", v=64),
                ps[:].rearrange("p (h v) -> p h v", v=64),
                colvec[:, g * 8:(g + 1) * 8].unsqueeze(2).broadcast_to((128, 8, 64)),
                OP.mult)
            return mg

        def scan(c):
            cs = slice(c * 128, (c + 1) * 128)
            h = H.pop(c)
            G, tdK, cbc, cc, dnrS = h["G"], h["tdK"], h["cbc"], h["cc"], h["dnrS"]
            # ---- KW + X0 = t1 - cbc*KW
            X0 = x0_p.tile([128, 1024], BF16, tag="x0", name="x0")
            for g in range(2):
                ps = psS_p.tile([128, 512], F32, tag="ss", name="ss")
                for j in range(8):
                    hd = g * 8 + j
                    b = hd * GW
                    for dc in range(2):
                        nc.tensor.matmul(ps[:, j * 64:(j + 1) * 64],
                                         G[dc][:, b + 128:b + 256],
                                         t_Wb[:, hd * 128 + dc * 64:
                                              hd * 128 + (dc + 1) * 64],
                                         start=(dc == 0), stop=(dc == 1))
                mg = bscale(ps, cbc, g)
                nc.vector.tensor_sub(X0[:, g * 512:(g + 1) * 512],
                                     h["t1"][:, g * 512:(g + 1) * 512], mg[:])
            # ---- Horner iterations: Z <- X0 - cbc*(NT^T Z)
            Zin = X0
            Zs = []
            for it in range(NHORN):
                Zout = z_p.tile([128, 1024], BF16, tag=f"z{it}", name=f"z{it}")
                for g in range(2):
                    ps = psS_p.tile([128, 512], F32, tag="ss", name="ss")
                    for j in range(8):
                        hd = g * 8 + j
                        nc.tensor.matmul(ps[:, j * 64:(j + 1) * 64],
                                         h["NT"][:, hd * 128:(hd + 1) * 128],
                                         Zin[:, hd * 64:(hd + 1) * 64],
                                         start=True, stop=True)
                    mg = bscale(ps, cbc, g)
                    nc.vector.tensor_sub(Zout[:, g * 512:(g + 1) * 512],
                                         X0[:, g * 512:(g + 1) * 512], mg[:])
                Zin = Zout
            Y = Zin
            # ---- out = (QW + tril_incl(S1) Y) * dnrS
            outc = oc_p.tile([128, 1024], BF16, tag="oc", name="oc")
            for g in range(2):
                ps = psS_p.tile([128, 512], F32, tag="ss", name="ss")
                for j in range(8):
                    hd = g * 8 + j
                    b = hd * GW
                    for dc in range(2):
                        nc.tensor.matmul(ps[:, j * 64:(j + 1) * 64],
                                         G[dc][:, b:b + 128],
                                         t_Wb[:, hd * 128 + dc * 64:
                                              hd * 128 + (dc + 1) * 64],
                                         start=(dc == 0), stop=False)
                    nc.tensor.matmul(ps[:, j * 64:(j + 1) * 64],
                                     h["Shat"][:, hd * 128:(hd + 1) * 128],
                                     Y[:, hd * 64:(hd + 1) * 64],
                                     start=False, stop=True)
                mg = bscale(ps, dnrS, g)
                nc.vector.tensor_copy(outc[:, g * 512:(g + 1) * 512], mg[:])
            # ---- transpose outc -> outT [i, t]
            outT = oT_p.tile([128, 1024], BF16, tag="oT", name="oT")
            for bk in range(2):
                pt = psT_p.tile([128, 512], BF16, tag="tp", name="tp")
                for j in range(8):
                    hd = bk * 8 + j
                    base = (hd % 2) * 64
                    nc.tensor.transpose(
                        pt[base:base + 64, (j // 2) * 128:(j // 2 + 1) * 128],
                        outc[:, hd * 64:(hd + 1) * 64], t_id[:],
                        tile_position=(0, base))
                nc.vector.tensor_copy(outT[:, bk * 512:(bk + 1) * 512], pt[:])
            # ---- W state update: Wm += tdK^T Y ; Wb = bf16(Wm)
            for rr in range(4):
                ps = psS_p.tile([128, 512], F32, tag="ss", name="ss")
                for j in range(4):
                    hd = rr * 4 + j
                    for dc in range(2):
                        nc.tensor.matmul(
                            ps[:, j * 128 + dc * 64:j * 128 + (dc + 1) * 64],
                            tdK[:, hd * 256 + dc * 128:hd * 256 + (dc + 1) * 128],
                            Y[:, hd * 64:(hd + 1) * 64], start=True, stop=True)
                nc.vector.tensor_add(t_Wm[:, rr * 512:(rr + 1) * 512], ps[:],
                                     t_Wm[:, rr * 512:(rr + 1) * 512])
                nc.scalar.copy(t_Wb[:, rr * 512:(rr + 1) * 512],
                               t_Wm[:, rr * 512:(rr + 1) * 512])
            # ---- output projection + residual + LN
            x = x_p.tile([128, DM], F32, tag="x", name="x")
            for og in range(2):
                ps = psP_p.tile([128, 512], F32, tag="pp", name="pp")
                for ic in range(8):
                    nc.tensor.matmul(ps[:], outT[:, ic * 128:(ic + 1) * 128],
                                     t_wo[ic][:, og * 512:(og + 1) * 512],
                                     start=(ic == 0), stop=(ic == 7))
                nc.vector.tensor_add(x[:, og * 512:(og + 1) * 512], ps[:],
                                     h["hr"][:, og * 512:(og + 1) * 512])
            xsum = cols_p.tile([128, 1], F32, tag="xsum", name="xsum")
            nc.vector.tensor_reduce(xsum[:], x[:], AX.X, OP.add)
            nmu = cols_p.tile([128, 1], F32, tag="nmu", name="nmu")
            nc.vector.tensor_scalar_mul(nmu[:], xsum[:], -1.0 / DM)
            nc.vector.tensor_scalar_add(x[:], x[:], nmu[:])
            vscr = ft_p.tile([128, DM], BF16, tag="vscr", name="vscr")
            var = cols_p.tile([128, 1], F32, tag="var", name="var")
            nc.vector.scalar_tensor_tensor(vscr[:], x[:], 1.0, x[:],
                                           OP.mult, OP.mult, accum_out=var[:])
            vare = cols_p.tile([128, 1], F32, tag="vare", name="vare")
            nc.vector.tensor_scalar(vare[:], var[:], 1.0 / DM, float(LN_EPS),
                                    OP.mult, OP.add)
            sd = cols_p.tile([128, 1], F32, tag="sd", name="sd")
            nc.scalar.sqrt(sd[:], vare[:])
            rstd = cols_p.tile([128, 1], F32, tag="rstd", name="rstd")
            nc.vector.reciprocal(rstd[:], sd[:])
            nc.vector.scalar_tensor_tensor(x[:], x[:], rstd[:], t_lng[:],
                                           OP.mult, OP.mult)
            nc.vector.tensor_add(x[:], x[:], t_lnb[:])
            nc.sync.dma_start(d_out[cs, :], x[:])

        prepA(0)
        prepB(0)
        for c in range(n_chunks):
            if c + 1 < n_chunks:
                prepA(c + 1)
            scan(c)
            if c + 1 < n_chunks:
                prepB(c + 1)

    return nc


# ---------------------------------------------------------------- host side
def _prep_core_inputs(h_b, W_qkvb, W_o, ln_g, ln_b):
    bf16 = ml_dtypes.bfloat16
    hT = np.ascontiguousarray(h_b.T).astype(bf16)                  # [1024, 2048]
    Wr = W_qkvb.reshape(NH, 193, DM)
    wq = np.empty((DM, 3088), dtype=bf16)
    wq[:, 0:1024] = Wr[:, 64:128, :].reshape(1024, DM).T           # K
    wq[:, 1024:2048] = Wr[:, 0:64, :].reshape(1024, DM).T          # Q
    wq[:, 2048:3072] = Wr[:, 128:192, :].reshape(1024, DM).T       # V
    wq[:, 3072:3088] = Wr[:, 192, :].T                             # b
    woT = np.ascontiguousarray(W_o.T).astype(bf16)                 # [i, o]
    lng = np.broadcast_to(ln_g[None, :], (128, DM)).astype(bf16).copy()
    lnb = np.broadcast_to(ln_b[None, :], (128, DM)).astype(bf16).copy()
    ii, jj = np.indices((128, 132))
    mSL = (jj < ii).astype(bf16);  mSL[:, 128] = 1.0
    mLI = (jj <= ii).astype(bf16); mLI[:, 128] = 1.0
    ii, jj = np.indices((128, 128))
    mUI = (jj >= ii).astype(bf16)
    mUS = (jj > ii).astype(bf16)
    identb = np.eye(128, dtype=bf16)
    return {"hT": hT, "hres": h_b.astype(bf16),
            "wqkv": wq, "woT": woT, "lng": lng, "lnb": lnb,
            "maskSL": mSL, "maskLI": mLI, "maskUI": mUI, "maskUS": mUS,
            "identb": identb}


_cached = {}


def kernel(h, W_qkvb, W_o, ln_g, ln_b):
    h = np.asarray(h, np.float32)
    W_qkvb = np.asarray(W_qkvb, np.float32)
    W_o = np.asarray(W_o, np.float32)
    ln_g = np.asarray(ln_g, np.float32)
    ln_b = np.asarray(ln_b, np.float32)
    if "nc" not in _cached:
        _cached["nc"] = build_program()
    nc = _cached["nc"]
    in_maps = [_prep_core_inputs(h[:, b, :], W_qkvb, W_o, ln_g, ln_b)
               for b in range(BSZ)]
    res = run_bass_kernel_spmd(nc, in_maps, list(range(BSZ)),
                               trace=os.environ.get("BASS_TRACE", "") == "1")
    out = np.stack([res.results[b]["out"] for b in range(BSZ)], axis=1)
    kernel.last_exec_time_ns = res.exec_time_ns
    return out.astype(np.float32)


# revision 11
# speedup vs baseline: 92.6804x; 1.2439x over previous
"""Trainium2 Bass kernel for the CudaNorm FastWeight DPFP transformer layer.

Sharding: batch (8) across the 8 cores; each core runs its batch's full layer.

v3: head-batched DVE/Act ops via 3D strided APs, [K|Q|V|b] projection layout,
interleaved per-head [Q|K] feature tiles so one SBUF->SBUF DMA transpose per
dc produces the whole feature-major G tile, kd/dn via PE column-sum matmuls
accumulated onto K.r/Q.r psum groups, unnormalized-Q trick, depth-1 Horner
Neumann solve with batched cbc post-scaling, stage-major scan with 8-head
psum packing, prepA/scan/prepB software pipelining across chunks.
"""
import os
import numpy as np
import ml_dtypes

import concourse.bass as bass
import concourse.mybir as mybir
from concourse.bass_utils import run_bass_kernel_spmd
from concourse.tile import TileContext
from concourse.vector_clock import ScopedClock, VectorClock
from contextlib import ExitStack

F32 = mybir.dt.float32
BF16 = mybir.dt.bfloat16
AF = mybir.ActivationFunctionType
OP = mybir.AluOpType
AX = mybir.AxisListType

SLEN, BSZ, DM = 2048, 8, 1024
NH, DH, NROLL = 16, 64, 2
D = 2 * NROLL * DH            # 256 feature dim
C = 128                       # chunk length
NCH = SLEN // C               # 16 chunks
EPS, LN_EPS = 1e-5, 1e-5
SCALE = 1.0 / float(np.sqrt(DH))
NHORN = 1                     # Horner/Neumann solve iterations

# ---------------------------------------------------------------- tile ctx
MAXW = 2


class PatchedTileContext(TileContext):
    """Work around walrus TPB sync-command limits: each instruction carries at
    most 2 sync commands (waits+updates); hoist excess waits onto preceding
    same-engine NoOps (1 wait each), and emit the kernel-tail drain's waits
    one-per-nop on SP."""

    def _lower_ordered_insts(self, ordered):
        for bb_name in list(ordered.keys()):
            new = []
            for inst in ordered[bb_name]:
                si = inst.sync_info
                nupd = len(si.on_update) if si is not None and si.on_update else 0
                maxw = max(0, MAXW - nupd)
                if si is not None and si.on_wait and len(si.on_wait) > maxw:
                    waits = list(si.on_wait)
                    excess = waits if maxw == 0 else waits[:-maxw]
                    keep = [] if maxw == 0 else waits[-maxw:]
                    for w in excess:
                        nop = mybir.InstNoOp(
                            name=self.nc.get_next_instruction_name(),
                            engine=inst.engine, ins=[], outs=[])
                        nop.sync_info = mybir.SyncInfo(on_wait=[w], on_update=[])
                        new.append(nop)
                    inst.sync_info = mybir.SyncInfo(
                        on_wait=keep, on_update=list(si.on_update or []))
                new.append(inst)
            ordered[bb_name] = new
        return super()._lower_ordered_insts(ordered)

    def _drain_and_barrier(self, tick_clock, wait_clock):
        gc = tick_clock.global_clock
        n = len(gc)
        for p in range(n):
            if gc[p] > 0:
                vc = VectorClock([gc[i] if i == p else 0 for i in range(n)])
                nop = self.nc.sync.nop(nofuse=True)
                wait_clock.add_sem_waits(nop.ins, ScopedClock({None: vc}))
        self.nc.sync.drain()
        self.nc.all_engine_barrier()
        assert self.sems is not None
        popped = self.nc._tile_sem_poison_stack.pop()
        assert popped is self._sem_poison
        self.nc.clear_and_free_semaphores(list(self.sems.allocated().values()))
        self.nc.all_engine_barrier()


# ---------------------------------------------------------------- program
def build_program(n_chunks=NCH):
    nc = bass.Bass()
    d_hT = nc.declare_dram_parameter("hT", [DM, SLEN], BF16, isOutput=False)
    d_hres = nc.declare_dram_parameter("hres", [SLEN, DM], BF16, isOutput=False)
    d_w = nc.declare_dram_parameter("wqkv", [DM, 3088], BF16, isOutput=False)
    d_wo = nc.declare_dram_parameter("woT", [DM, DM], BF16, isOutput=False)
    d_lng = nc.declare_dram_parameter("lng", [128, DM], BF16, isOutput=False)
    d_lnb = nc.declare_dram_parameter("lnb", [128, DM], BF16, isOutput=False)
    d_mUI = nc.declare_dram_parameter("maskUI", [128, 128], BF16, isOutput=False)
    d_mUS = nc.declare_dram_parameter("maskUS", [128, 128], BF16, isOutput=False)
    d_out = nc.declare_dram_parameter("out", [SLEN, DM], F32, isOutput=True)

    with PatchedTileContext(nc) as tc, ExitStack() as ctx:
        P = lambda name, bufs, **kw: ctx.enter_context(
            tc.tile_pool(name=name, bufs=bufs, **kw))
        const = P("const", 1)
        state = P("state", 1)
        hts_p = P("hts", 2)
        xp_p = P("xp", 1)
        f_p = P("f", 2)
        G_p = P("G", 2)
        V_p = P("V", 1)
        t1_p = P("t1", 2)
        cols_p = P("cols", 2)
        sa_p = P("sa", 3)
        sh_p = P("sh", 2)
        nt_p = P("nt", 2)
        x0_p = P("x0", 1)
        z_p = P("z", 1)
        mg_p = P("mg", 3)
        oc_p = P("oc", 1)
        oT_p = P("oT", 1)
        hr_p = P("hr", 2)
        x_p = P("x", 2)
        psP_p = P("psP", 2, space="PSUM")
        psG_p = P("psG", 3, space="PSUM")
        psS_p = P("psS", 3, space="PSUM")

        # ---- constants
        t_mUI = const.tile([128, 128], BF16, tag="mUI", name="mUI"); nc.sync.dma_start(t_mUI[:], d_mUI[:])
        t_mUS = const.tile([128, 128], BF16, tag="mUS", name="mUS"); nc.sync.dma_start(t_mUS[:], d_mUS[:])
        t_lng = const.tile([128, DM], BF16, tag="lng", name="lng"); nc.sync.dma_start(t_lng[:], d_lng[:])
        t_lnb = const.tile([128, DM], BF16, tag="lnb", name="lnb"); nc.sync.dma_start(t_lnb[:], d_lnb[:])
        t_ones = const.tile([128, 1], BF16, tag="ones", name="ones")
        nc.vector.memset(t_ones[:], 1.0)
        t_w = []
        for mc in range(8):
            t = const.tile([128, 3088], BF16, tag=f"w{mc}", name=f"w{mc}")
            nc.sync.dma_start(t[:], d_w[mc * 128:(mc + 1) * 128, :])
            t_w.append(t)
        t_wo = []
        for ic in range(8):
            t = const.tile([128, DM], BF16, tag=f"wo{ic}", name=f"wo{ic}")
            nc.sync.dma_start(t[:], d_wo[ic * 128:(ic + 1) * 128, :])
            t_wo.append(t)

        # ---- state
        t_Wm = state.tile([128, NH * 128], F32, tag="Wm", name="Wm")
        nc.vector.memset(t_Wm[:], 0.0)
        t_Wb = state.tile([128, NH * 128], BF16, tag="Wb", name="Wb")
        nc.vector.memset(t_Wb[:], 0.0)
        t_r, t_rb = [], []
        for dc in range(2):
            r = state.tile([128, NH], F32, tag=f"r{dc}", name=f"r{dc}")
            nc.vector.memset(r[:], 0.0)
            t_r.append(r)
            rb = state.tile([128, NH], BF16, tag=f"rb{dc}", name=f"rb{dc}")
            nc.vector.memset(rb[:], 0.0)
            t_rb.append(rb)

        H = {}

        def prepA(c):
            cs = slice(c * 128, (c + 1) * 128)
            hts = hts_p.tile([128, 1024], BF16, tag="hts", name="hts")
            nc.sync.dma_start(
                hts[:].rearrange("p (mc t) -> p mc t", t=128),
                d_hT[:, cs].rearrange("(mc p) t -> p mc t", p=128))
            hr = hr_p.tile([128, DM], BF16, tag="hr", name="hr")
            nc.sync.dma_start(hr[:], d_hres[cs, :])
            xpK = xp_p.tile([128, 2048], BF16, tag="xpK", name="xpK")
            xpQ = xp_p.tile([128, 2048], BF16, tag="xpQ", name="xpQ")
            Vall = V_p.tile([128, 1024], BF16, tag="V", name="V")
            sig = cols_p.tile([128, NH], F32, tag="sig", name="sig")
            for og in range(6):
                ps = psP_p.tile([128, 512], F32, tag="pp", name="pp")
                for mc in range(8):
                    nc.tensor.matmul(ps[:], hts[:, mc * 128:(mc + 1) * 128],
                                     t_w[mc][:, og * 512:(og + 1) * 512],
                                     start=(mc == 0), stop=(mc == 7))
                psr = ps[:].rearrange("p (h v) -> p h v", v=64)
                if og < 2:      # K heads 8*og..8*og+7
                    xr = xpK[:].rearrange("p (h w) -> p h w", w=128)
                    nc.scalar.activation(xr[:, og * 8:(og + 1) * 8, 0:64], psr, AF.Relu)
                    nc.scalar.activation(xr[:, og * 8:(og + 1) * 8, 64:128], psr,
                                         AF.Relu, scale=-1.0)
                elif og < 4:    # Q heads
                    xr = xpQ[:].rearrange("p (h w) -> p h w", w=128)
                    hq = og - 2
                    nc.scalar.activation(xr[:, hq * 8:(hq + 1) * 8, 0:64], psr, AF.Relu)
                    nc.scalar.activation(xr[:, hq * 8:(hq + 1) * 8, 64:128], psr,
                                         AF.Relu, scale=-1.0)
                else:           # V
                    gv = og - 4
                    nc.scalar.copy(Vall[:, gv * 512:(gv + 1) * 512], ps[:])
            psb = psP_p.tile([128, NH], F32, tag="pp", name="pp")
            for mc in range(8):
                nc.tensor.matmul(psb[:], hts[:, mc * 128:(mc + 1) * 128],
                                 t_w[mc][:, 3072:3088],
                                 start=(mc == 0), stop=(mc == 7))
            nc.scalar.activation(sig[:], psb[:], AF.Sigmoid)
            H[c] = dict(xpK=xpK, xpQ=xpQ, Vall=Vall, sig=sig, hr=hr)

        def prepB(c):
            h = H[c]
            # f tiles: per dc, per head [Q-roll-dc 128 | K-roll-dc 128]
            f = [f_p.tile([128, 4096], BF16, tag=f"f{dc}", name=f"f{dc}")
                 for dc in range(2)]
            fr = [f[dc][:].rearrange("p (h w) -> p h w", w=256)
                  for dc in range(2)]
            for side, xp in ((0, h["xpQ"]), (1, h["xpK"])):
                xr = xp[:].rearrange("p (h w) -> p h w", w=128)
                o = side * 128
                for dc in range(2):
                    rl = dc + 1
                    nc.vector.tensor_mul(fr[dc][:, :, o + rl:o + 128],
                                         xr[:, :, rl:128], xr[:, :, 0:128 - rl])
                    nc.vector.tensor_mul(fr[dc][:, :, o:o + rl],
                                         xr[:, :, 0:rl], xr[:, :, 128 - rl:128])
            # feature sums (fp32) + reciprocals; K normalized in place, Q not
            frec = []
            for side in range(2):
                o = side * 128
                s0 = cols_p.tile([128, NH], F32, tag=f"s0{side}", name=f"s0{side}")
                s1 = cols_p.tile([128, NH], F32, tag=f"s1{side}", name=f"s1{side}")
                nc.vector.tensor_reduce(s0[:], fr[0][:, :, o:o + 128], AX.X, OP.add)
                nc.vector.tensor_reduce(s1[:], fr[1][:, :, o:o + 128], AX.X, OP.add)
                fs = cols_p.tile([128, NH], F32, tag=f"fs{side}", name=f"fs{side}")
                nc.vector.tensor_add(fs[:], s0[:], s1[:])
                fc = cols_p.tile([128, NH], F32, tag=f"fc{side}", name=f"fc{side}")
                nc.vector.reciprocal(fc[:], fs[:])
                frec.append(fc)
            frecQ, frecK = frec[0], frec[1]
            for dc in range(2):
                nc.vector.tensor_tensor(
                    fr[dc][:, :, 128:256], fr[dc][:, :, 128:256],
                    frecK[:].unsqueeze(2).broadcast_to((128, NH, 128)), OP.mult)
            # one DMA transpose per dc: G = f^T, per head [Q1T 128 | K1T 128]
            G = [G_p.tile([128, 4096], BF16, tag=f"G{dc}", name=f"G{dc}")
                 for dc in range(2)]
            for dc in range(2):
                nc.sync.dma_start_transpose(
                    G[dc][:].rearrange("p (b l) -> p b l", l=128), f[dc][:])
            # r state update AFTER gram Kr/Qr reads are emitted (see below);
            # grams per head: psum [S1 128 | A 128 | kd 1 | dn 1]
            kd = cols_p.tile([128, NH], F32, tag="kd", name="kd")
            dna = cols_p.tile([128, NH], F32, tag="dna", name="dna")
            Shat = sh_p.tile([128, NH * 128], BF16, tag="sh", name="sh")
            NT = nt_p.tile([128, NH * 128], BF16, tag="nt", name="nt")

            pend = []

            def gram_front(hd):
                ps = psG_p.tile([128, 258], F32, tag="gg", name="gg")
                b = hd * 256
                for dc in range(2):
                    nc.tensor.matmul(ps[:, 0:256], G[dc][:, b + 128:b + 256],
                                     G[dc][:, b:b + 256],
                                     start=(dc == 0), stop=(dc == 1))
                sa = sa_p.tile([128, 256], BF16, tag="sa", name="sa")
                nc.scalar.copy(sa[:], ps[:, 0:256])
                nc.gpsimd.tensor_mul(Shat[:, hd * 128:(hd + 1) * 128],
                                     sa[:, 0:128], t_mUI[:])
                nc.gpsimd.tensor_mul(NT[:, hd * 128:(hd + 1) * 128],
                                     sa[:, 128:256], t_mUS[:])
                return ps

            def gram_back(hd, ps):
                b = hd * 256
                for dc in range(2):
                    nc.tensor.matmul(ps[:, 256:257], G[dc][:, b + 128:b + 256],
                                     t_rb[dc][:, hd:hd + 1],
                                     start=(dc == 0), stop=False)
                nc.tensor.matmul(ps[:, 256:257], NT[:, hd * 128:(hd + 1) * 128],
                                 t_ones[:], start=False, stop=True)
                for dc in range(2):
                    nc.tensor.matmul(ps[:, 257:258], G[dc][:, b:b + 128],
                                     t_rb[dc][:, hd:hd + 1],
                                     start=(dc == 0), stop=False)
                nc.tensor.matmul(ps[:, 257:258], Shat[:, hd * 128:(hd + 1) * 128],
                                 t_ones[:], start=False, stop=True)
                nc.vector.tensor_copy(kd[:, hd:hd + 1], ps[:, 256:257])
                nc.vector.tensor_copy(dna[:, hd:hd + 1], ps[:, 257:258])

            LAG = 2
            for hd in range(NH):
                pend.append((hd, gram_front(hd)))
                if hd >= LAG:
                    gram_back(*pend[hd - LAG])
            for i in range(NH - LAG, NH):
                gram_back(*pend[i])
            if c == 0:
                nc.vector.memset(kd[0:1, :], 1.0)
            # r += per-head column sums of K1T (feature-major K in G)
            for dc in range(2):
                rs = cols_p.tile([128, NH], F32, tag=f"rs{dc}", name=f"rs{dc}")
                nc.vector.tensor_reduce(
                    rs[:], G[dc][:].rearrange("p (h w) -> p h w", w=256)[:, :, 128:256],
                    AX.X, OP.add)
                nc.vector.tensor_add(t_r[dc][:], rs[:], t_r[dc][:])
                nc.scalar.copy(t_rb[dc][:], t_r[dc][:])

            # chunk-level columns
            ceps = cols_p.tile([128, NH], F32, tag="ceps", name="ceps")
            nc.vector.tensor_scalar_add(ceps[:], kd[:], EPS)
            cc = cols_p.tile([128, NH], F32, tag="cc", name="cc")
            nc.vector.reciprocal(cc[:], ceps[:])
            t0 = cols_p.tile([128, NH], F32, tag="t0", name="t0")
            nc.vector.tensor_mul(t0[:], kd[:], cc[:])
            cb = cols_p.tile([128, NH], F32, tag="cb", name="cb")
            nc.vector.tensor_mul(cb[:], t0[:], h["sig"][:])
            cbc = cols_p.tile([128, NH], F32, tag="cbc", name="cbc")
            nc.vector.tensor_mul(cbc[:], cb[:], cc[:])
            dnm = cols_p.tile([128, NH], F32, tag="dnm", name="dnm")
            nc.vector.tensor_mul(dnm[:], dna[:], frecQ[:])
            dne = cols_p.tile([128, NH], F32, tag="dne", name="dne")
            nc.vector.tensor_scalar_add(dne[:], dnm[:], EPS)
            dnr = cols_p.tile([128, NH], F32, tag="dnr", name="dnr")
            nc.vector.reciprocal(dnr[:], dne[:])
            dnr2 = cols_p.tile([128, NH], F32, tag="dnr2", name="dnr2")
            nc.vector.tensor_scalar_mul(dnr2[:], dnr[:], SCALE)
            dnrS = cols_p.tile([128, NH], F32, tag="dnrS", name="dnrS")
            nc.vector.tensor_mul(dnrS[:], dnr2[:], frecQ[:])
            # t1 = cb * V
            t1 = t1_p.tile([128, 1024], BF16, tag="t1", name="t1")
            nc.vector.tensor_tensor(
                t1[:].rearrange("p (h v) -> p h v", v=64),
                h["Vall"][:].rearrange("p (h v) -> p h v", v=64),
                cb[:].unsqueeze(2).broadcast_to((128, NH, 64)), OP.mult)
            h.update(G=G, f=f, Shat=Shat, NT=NT, cbc=cbc, dnrS=dnrS, t1=t1)

        def bscale(out, ps, colvec, g):
            """out (bf16) = psum [128,512] * per-head column broadcast."""
            nc.vector.tensor_tensor(
                out.rearrange("p (h v) -> p h v", v=64),
                ps[:].rearrange("p (h v) -> p h v", v=64),
                colvec[:, g * 8:(g + 1) * 8].unsqueeze(2).broadcast_to((128, 8, 64)),
                OP.mult)

        def scan(c):
            cs = slice(c * 128, (c + 1) * 128)
            h = H.pop(c)
            G, f, cbc, dnrS = h["G"], h["f"], h["cbc"], h["dnrS"]
            # ---- KW + X0 = t1 - cbc*KW
            X0 = x0_p.tile([128, 1024], BF16, tag="x0", name="x0")
            for g in range(2):
                ps = psS_p.tile([128, 512], F32, tag="ss", name="ss")
                for j in range(8):
                    hd = g * 8 + j
                    b = hd * 256
                    for dc in range(2):
                        nc.tensor.matmul(ps[:, j * 64:(j + 1) * 64],
                                         G[dc][:, b + 128:b + 256],
                                         t_Wb[:, hd * 128 + dc * 64:
                                              hd * 128 + (dc + 1) * 64],
                                         start=(dc == 0), stop=(dc == 1))
                mg = mg_p.tile([128, 512], BF16, tag="mg", name="mg")
                bscale(mg[:], ps, cbc, g)
                nc.vector.tensor_sub(X0[:, g * 512:(g + 1) * 512],
                                     h["t1"][:, g * 512:(g + 1) * 512], mg[:])
            # ---- Horner iterations: Z <- X0 - cbc*(NT^T Z)
            Zin = X0
            for it in range(NHORN):
                Zout = z_p.tile([128, 1024], BF16, tag=f"z{it}", name=f"z{it}")
                for g in range(2):
                    ps = psS_p.tile([128, 512], F32, tag="ss", name="ss")
                    for j in range(8):
                        hd = g * 8 + j
                        nc.tensor.matmul(ps[:, j * 64:(j + 1) * 64],
                                         h["NT"][:, hd * 128:(hd + 1) * 128],
                                         Zin[:, hd * 64:(hd + 1) * 64],
                                         start=True, stop=True)
                    mg = mg_p.tile([128, 512], BF16, tag="mg", name="mg")
                    bscale(mg[:], ps, cbc, g)
                    nc.vector.tensor_sub(Zout[:, g * 512:(g + 1) * 512],
                                         X0[:, g * 512:(g + 1) * 512], mg[:])
                Zin = Zout
            Y = Zin
            # ---- outc = (QW + tril_incl(S1) Y) * dnrS
            outc = oc_p.tile([128, 1024], BF16, tag="oc", name="oc")
            for g in range(2):
                ps = psS_p.tile([128, 512], F32, tag="ss", name="ss")
                for j in range(8):
                    hd = g * 8 + j
                    b = hd * 256
                    for dc in range(2):
                        nc.tensor.matmul(ps[:, j * 64:(j + 1) * 64],
                                         G[dc][:, b:b + 128],
                                         t_Wb[:, hd * 128 + dc * 64:
                                              hd * 128 + (dc + 1) * 64],
                                         start=(dc == 0), stop=False)
                    nc.tensor.matmul(ps[:, j * 64:(j + 1) * 64],
                                     h["Shat"][:, hd * 128:(hd + 1) * 128],
                                     Y[:, hd * 64:(hd + 1) * 64],
                                     start=False, stop=True)
                bscale(outc[:, g * 512:(g + 1) * 512], ps, dnrS, g)
            # ---- transpose outc -> outT [i, t] via one DMA
            outT = oT_p.tile([128, 1024], BF16, tag="oT", name="oT")
            nc.sync.dma_start_transpose(
                outT[:].rearrange("p (b l) -> p b l", l=128), outc[:])
            # ---- W state update: Wm += K1^T Y ; Wb = bf16(Wm)
            for rr in range(4):
                ps = psS_p.tile([128, 512], F32, tag="ss", name="ss")
                for j in range(4):
                    hd = rr * 4 + j
                    for dc in range(2):
                        nc.tensor.matmul(
                            ps[:, j * 128 + dc * 64:j * 128 + (dc + 1) * 64],
                            f[dc][:, hd * 256 + 128:hd * 256 + 256],
                            Y[:, hd * 64:(hd + 1) * 64], start=True, stop=True)
                nc.vector.tensor_add(t_Wm[:, rr * 512:(rr + 1) * 512], ps[:],
                                     t_Wm[:, rr * 512:(rr + 1) * 512])
                nc.scalar.copy(t_Wb[:, rr * 512:(rr + 1) * 512],
                               t_Wm[:, rr * 512:(rr + 1) * 512])
            # ---- output projection + residual + LN
            x = x_p.tile([128, DM], F32, tag="x", name="x")
            for og in range(2):
                ps = psP_p.tile([128, 512], F32, tag="pp", name="pp")
                for ic in range(8):
                    nc.tensor.matmul(ps[:], outT[:, ic * 128:(ic + 1) * 128],
                                     t_wo[ic][:, og * 512:(og + 1) * 512],
                                     start=(ic == 0), stop=(ic == 7))
                nc.vector.tensor_add(x[:, og * 512:(og + 1) * 512], ps[:],
                                     h["hr"][:, og * 512:(og + 1) * 512])
            xsum = cols_p.tile([128, 1], F32, tag="xsum", name="xsum")
            nc.vector.tensor_reduce(xsum[:], x[:], AX.X, OP.add)
            nmu = cols_p.tile([128, 1], F32, tag="nmu", name="nmu")
            nc.vector.tensor_scalar_mul(nmu[:], xsum[:], -1.0 / DM)
            nc.vector.tensor_scalar_add(x[:], x[:], nmu[:])
            vscr = oc_p.tile([128, DM], BF16, tag="oc", name="vscr")
            var = cols_p.tile([128, 1], F32, tag="var", name="var")
            nc.vector.scalar_tensor_tensor(vscr[:], x[:], 1.0, x[:],
                                           OP.mult, OP.mult, accum_out=var[:])
            vare = cols_p.tile([128, 1], F32, tag="vare", name="vare")
            nc.vector.tensor_scalar(vare[:], var[:], 1.0 / DM, float(LN_EPS),
                                    OP.mult, OP.add)
            sd = cols_p.tile([128, 1], F32, tag="sd", name="sd")
            nc.scalar.sqrt(sd[:], vare[:])
            rstd = cols_p.tile([128, 1], F32, tag="rstd", name="rstd")
            nc.vector.reciprocal(rstd[:], sd[:])
            nc.vector.scalar_tensor_tensor(x[:], x[:], rstd[:], t_lng[:],
                                           OP.mult, OP.mult)
            nc.vector.tensor_add(x[:], x[:], t_lnb[:])
            nc.sync.dma_start(d_out[cs, :], x[:])

        prepA(0)
        prepB(0)
        for c in range(n_chunks):
            if c + 1 < n_chunks:
                prepA(c + 1)
            scan(c)
            if c + 1 < n_chunks:
                prepB(c + 1)

    return nc


# ---------------------------------------------------------------- host side
def _prep_core_inputs(h_b, W_qkvb, W_o, ln_g, ln_b):
    bf16 = ml_dtypes.bfloat16
    hT = np.ascontiguousarray(h_b.T).astype(bf16)                  # [1024, 2048]
    Wr = W_qkvb.reshape(NH, 193, DM)
    wq = np.empty((DM, 3088), dtype=bf16)
    wq[:, 0:1024] = Wr[:, 64:128, :].reshape(1024, DM).T           # K
    wq[:, 1024:2048] = Wr[:, 0:64, :].reshape(1024, DM).T          # Q
    wq[:, 2048:3072] = Wr[:, 128:192, :].reshape(1024, DM).T       # V
    wq[:, 3072:3088] = Wr[:, 192, :].T                             # b
    woT = np.ascontiguousarray(W_o.T).astype(bf16)                 # [i, o]
    lng = np.broadcast_to(ln_g[None, :], (128, DM)).astype(bf16).copy()
    lnb = np.broadcast_to(ln_b[None, :], (128, DM)).astype(bf16).copy()
    ii, jj = np.indices((128, 128))
    mUI = (jj >= ii).astype(bf16)
    mUS = (jj > ii).astype(bf16)
    return {"hT": hT, "hres": h_b.astype(bf16),
            "wqkv": wq, "woT": woT, "lng": lng, "lnb": lnb,
            "maskUI": mUI, "maskUS": mUS}


_cached = {}


def kernel(h, W_qkvb, W_o, ln_g, ln_b):
    h = np.asarray(h, np.float32)
    W_qkvb = np.asarray(W_qkvb, np.float32)
    W_o = np.asarray(W_o, np.float32)
    ln_g = np.asarray(ln_g, np.float32)
    ln_b = np.asarray(ln_b, np.float32)
    if "nc" not in _cached:
        _cached["nc"] = build_program()
    nc = _cached["nc"]
    in_maps = [_prep_core_inputs(h[:, b, :], W_qkvb, W_o, ln_g, ln_b)
               for b in range(BSZ)]
    res = run_bass_kernel_spmd(nc, in_maps, list(range(BSZ)),
                               trace=os.environ.get("BASS_TRACE", "") == "1")
    out = np.stack([res.results[b]["out"] for b in range(BSZ)], axis=1)
    kernel.last_exec_time_ns = res.exec_time_ns
    return out.astype(np.float32)


# revision 35
# speedup vs baseline: 101.6032x; 1.0963x over previous
"""Trainium2 Bass kernel for the CudaNorm FastWeight DPFP transformer layer.

Sharding: batch (8) across the 8 cores; each core runs its batch's full layer.

v3: head-batched DVE/Act ops via 3D strided APs, [K|Q|V|b] projection layout,
interleaved per-head [Q|K] feature tiles so one SBUF->SBUF DMA transpose per
dc produces the whole feature-major G tile, kd/dn via PE column-sum matmuls
accumulated onto K.r/Q.r psum groups, unnormalized-Q trick, depth-1 Horner
Neumann solve with batched cbc post-scaling, stage-major scan with 8-head
psum packing, prepA/scan/prepB software pipelining across chunks.
"""
import os
import numpy as np
import ml_dtypes

import concourse.bass as bass
import concourse.mybir as mybir
from concourse.bass_utils import run_bass_kernel_spmd
from concourse.tile import TileContext
from concourse.vector_clock import ScopedClock, VectorClock
from contextlib import ExitStack

F32 = mybir.dt.float32
BF16 = mybir.dt.bfloat16
AF = mybir.ActivationFunctionType
OP = mybir.AluOpType
AX = mybir.AxisListType

SLEN, BSZ, DM = 2048, 8, 1024
NH, DH, NROLL = 16, 64, 2
D = 2 * NROLL * DH            # 256 feature dim
C = 128                       # chunk length
NCH = SLEN // C               # 16 chunks
EPS, LN_EPS = 1e-5, 1e-5
SCALE = 1.0 / float(np.sqrt(DH))
NHORN = 1                     # Horner/Neumann solve iterations

# ---------------------------------------------------------------- tile ctx
MAXW = 2


class PatchedTileContext(TileContext):
    """Work around walrus TPB sync-command limits: each instruction carries at
    most 2 sync commands (waits+updates); hoist excess waits onto preceding
    same-engine NoOps (1 wait each), and emit the kernel-tail drain's waits
    one-per-nop on SP."""

    def _lower_ordered_insts(self, ordered):
        for bb_name in list(ordered.keys()):
            new = []
            for inst in ordered[bb_name]:
                si = inst.sync_info
                nupd = len(si.on_update) if si is not None and si.on_update else 0
                maxw = max(0, MAXW - nupd)
                if si is not None and si.on_wait and len(si.on_wait) > maxw:
                    waits = list(si.on_wait)
                    excess = waits if maxw == 0 else waits[:-maxw]
                    keep = [] if maxw == 0 else waits[-maxw:]
                    for w in excess:
                        nop = mybir.InstNoOp(
                            name=self.nc.get_next_instruction_name(),
                            engine=inst.engine, ins=[], outs=[])
                        nop.sync_info = mybir.SyncInfo(on_wait=[w], on_update=[])
                        new.append(nop)
                    inst.sync_info = mybir.SyncInfo(
                        on_wait=keep, on_update=list(si.on_update or []))
                new.append(inst)
            ordered[bb_name] = new
        return super()._lower_ordered_insts(ordered)

    def _drain_and_barrier(self, tick_clock, wait_clock):
        gc = tick_clock.global_clock
        n = len(gc)
        for p in range(n):
            if gc[p] > 0:
                vc = VectorClock([gc[i] if i == p else 0 for i in range(n)])
                nop = self.nc.sync.nop(nofuse=True)
                wait_clock.add_sem_waits(nop.ins, ScopedClock({None: vc}))
        self.nc.sync.drain()
        self.nc.all_engine_barrier()
        assert self.sems is not None
        popped = self.nc._tile_sem_poison_stack.pop()
        assert popped is self._sem_poison
        self.nc.clear_and_free_semaphores(list(self.sems.allocated().values()))
        self.nc.all_engine_barrier()


# ---------------------------------------------------------------- program
def build_program(n_chunks=NCH):
    nc = bass.Bass()
    d_hT = nc.declare_dram_parameter("hT", [DM, SLEN], BF16, isOutput=False)
    d_hres = nc.declare_dram_parameter("hres", [SLEN, DM], BF16, isOutput=False)
    d_w = nc.declare_dram_parameter("wqkv", [DM, 3088], BF16, isOutput=False)
    d_wo = nc.declare_dram_parameter("woT", [DM, DM], BF16, isOutput=False)
    d_lng = nc.declare_dram_parameter("lng", [128, DM], BF16, isOutput=False)
    d_lnb = nc.declare_dram_parameter("lnb", [128, DM], BF16, isOutput=False)
    d_mUI = nc.declare_dram_parameter("maskUI", [128, 128], BF16, isOutput=False)
    d_mUS = nc.declare_dram_parameter("maskUS", [128, 128], BF16, isOutput=False)
    d_out = nc.declare_dram_parameter("out", [SLEN, DM], BF16, isOutput=True)

    with PatchedTileContext(nc) as tc, ExitStack() as ctx:
        P = lambda name, bufs, **kw: ctx.enter_context(
            tc.tile_pool(name=name, bufs=bufs, **kw))
        const = P("const", 1)
        state = P("state", 1)
        hts_p = P("hts", 2)
        xp_p = P("xp", 1)
        f_p = P("f", 2)
        G_p = P("G", 2)
        V_p = P("V", 1)
        t1_p = P("t1", 2)
        cols_p = P("cols", 2)
        sa_p = P("sa", 2)
        sh_p = P("sh", 2)
        nt_p = P("nt", 2)
        x0_p = P("x0", 1)
        z_p = P("z", 1)
        mg_p = P("mg", 2)
        oc_p = P("oc", 1)
        oT_p = P("oT", 1)
        hr_p = P("hr", 2)
        x_p = P("x", 2)
        psP_p = P("psP", 2, space="PSUM")
        psG_p = P("psG", 3, space="PSUM")
        psS_p = P("psS", 3, space="PSUM")

        # ---- constants
        t_mUI = const.tile([128, 128], BF16, tag="mUI", name="mUI"); nc.sync.dma_start(t_mUI[:], d_mUI[:])
        t_mUS = const.tile([128, 128], BF16, tag="mUS", name="mUS"); nc.sync.dma_start(t_mUS[:], d_mUS[:])
        t_lng = const.tile([128, DM], BF16, tag="lng", name="lng"); nc.sync.dma_start(t_lng[:], d_lng[:])
        t_lnb = const.tile([128, DM], BF16, tag="lnb", name="lnb"); nc.sync.dma_start(t_lnb[:], d_lnb[:])
        t_ones = const.tile([128, 1], BF16, tag="ones", name="ones")
        nc.vector.memset(t_ones[:], 1.0)
        t_w = []
        for mc in range(8):
            t = const.tile([128, 3088], BF16, tag=f"w{mc}", name=f"w{mc}")
            nc.sync.dma_start(t[:], d_w[mc * 128:(mc + 1) * 128, :])
            t_w.append(t)
        t_wo = []
        for ic in range(8):
            t = const.tile([128, DM], BF16, tag=f"wo{ic}", name=f"wo{ic}")
            nc.sync.dma_start(t[:], d_wo[ic * 128:(ic + 1) * 128, :])
            t_wo.append(t)

        # ---- state
        t_Wm = state.tile([128, NH * 128], F32, tag="Wm", name="Wm")
        nc.vector.memset(t_Wm[:], 0.0)
        t_Wb = state.tile([128, NH * 128], BF16, tag="Wb", name="Wb")
        nc.vector.memset(t_Wb[:], 0.0)
        t_r, t_rb = [], []
        for dc in range(2):
            r = state.tile([128, NH], F32, tag=f"r{dc}", name=f"r{dc}")
            nc.vector.memset(r[:], 0.0)
            t_r.append(r)
            rb = state.tile([128, NH], BF16, tag=f"rb{dc}", name=f"rb{dc}")
            nc.vector.memset(rb[:], 0.0)
            t_rb.append(rb)
        # Y with per-head stride 65; col 64 stays 1.0 so the pW matmuls also
        # produce the per-chunk r increment (sum_t K[t,f]) for free.
        t_Y = state.tile([128, NH * 65], BF16, tag="Y65", name="Y65")
        nc.vector.memset(t_Y[:], 1.0)

        H = {}

        def prepA(c):
            cs = slice(c * 128, (c + 1) * 128)
            hts = hts_p.tile([128, 1024], BF16, tag="hts", name="hts")
            nc.sync.dma_start(
                hts[:].rearrange("p (mc t) -> p mc t", t=128),
                d_hT[:, cs].rearrange("(mc p) t -> p mc t", p=128))
            hr = hr_p.tile([128, DM], BF16, tag="hr", name="hr")
            nc.sync.dma_start(hr[:], d_hres[cs, :])
            # xpC: per head [relu(Q) 64 | relu(-Q) 64 | relu(K) 64 | relu(-K) 64]
            xpC = xp_p.tile([128, 4096], BF16, tag="xpC", name="xpC")
            xr = xpC[:].rearrange("p (h w) -> p h w", w=256)
            Vall = V_p.tile([128, 1024], BF16, tag="V", name="V")
            sig = cols_p.tile([128, NH], F32, tag="sig", name="sig")
            for og in range(6):
                ps = psP_p.tile([128, 512], F32, tag="pp", name="pp")
                for mc in range(8):
                    nc.tensor.matmul(ps[:], hts[:, mc * 128:(mc + 1) * 128],
                                     t_w[mc][:, og * 512:(og + 1) * 512],
                                     start=(mc == 0), stop=(mc == 7))
                psr = ps[:].rearrange("p (h v) -> p h v", v=64)
                if og < 4:      # K (og 0,1) / Q (og 2,3), heads 8*(og%2)..
                    o = 128 if og < 2 else 0
                    hs = slice((og % 2) * 8, (og % 2) * 8 + 8)
                    nc.scalar.activation(xr[:, hs, o:o + 64], psr, AF.Relu)
                    nc.scalar.activation(xr[:, hs, o + 64:o + 128], psr,
                                         AF.Relu, scale=-1.0)
                else:           # V
                    gv = og - 4
                    nc.scalar.copy(Vall[:, gv * 512:(gv + 1) * 512], ps[:])
            psb = psP_p.tile([128, NH], F32, tag="pp", name="pp")
            for mc in range(8):
                nc.tensor.matmul(psb[:], hts[:, mc * 128:(mc + 1) * 128],
                                 t_w[mc][:, 3072:3088],
                                 start=(mc == 0), stop=(mc == 7))
            nc.scalar.activation(sig[:], psb[:], AF.Sigmoid)
            H[c] = dict(xpC=xpC, Vall=Vall, sig=sig, hr=hr)

        def prepB1(c):
            h = H[c]
            # f tiles: per dc, per head [Q-roll-dc 128 | K-roll-dc 128]
            f = [f_p.tile([128, 4096], BF16, tag=f"f{dc}", name=f"f{dc}")
                 for dc in range(2)]
            fr = [f[dc][:].rearrange("p (h w) -> p h w", w=256)
                  for dc in range(2)]
            xpC = h["xpC"]
            xr = xpC[:].rearrange("p (h w) -> p h w", w=256)
            for dc in range(2):
                rl = dc + 1
                for o in (0, 128):  # Q block, K block per head
                    nc.vector.tensor_mul(fr[dc][:, :, o + rl:o + 128],
                                         xr[:, :, o + rl:o + 128],
                                         xr[:, :, o:o + 128 - rl])
                    nc.vector.tensor_mul(fr[dc][:, :, o:o + rl],
                                         xr[:, :, o:o + rl],
                                         xr[:, :, o + 128 - rl:o + 128])
            # feature sums: packed-2x bf16 pair-adds fold 256 values down to
            # 64 per head, then one small fp32 reduce
            frec = []
            for side in range(2):
                o = side * 128
                b1 = V_p.tile([128, 1024], BF16, tag="b1", name="b1")
                b2 = V_p.tile([128, 1024], BF16, tag="b2", name="b2")
                b1r = b1[:].rearrange("p (h w) -> p h w", w=64)
                b2r = b2[:].rearrange("p (h w) -> p h w", w=64)
                nc.vector.tensor_add(b1r, fr[0][:, :, o:o + 64],
                                     fr[0][:, :, o + 64:o + 128])
                nc.vector.tensor_add(b2r, fr[1][:, :, o:o + 64],
                                     fr[1][:, :, o + 64:o + 128])
                nc.vector.tensor_add(b1[:], b1[:], b2[:])
                fs = cols_p.tile([128, NH], F32, tag=f"fs{side}", name=f"fs{side}")
                nc.vector.tensor_reduce(fs[:], b1r, AX.X, OP.add)
                fc = cols_p.tile([128, NH], F32, tag=f"fc{side}", name=f"fc{side}")
                nc.vector.reciprocal(fc[:], fs[:])
                frec.append(fc)
            frecQ, frecK = frec[0], frec[1]
            # materialize the frecK broadcast on Act so the DVE muls run packed
            fkb = V_p.tile([128, NH * 128], BF16, tag="fkb", name="fkb")
            nc.scalar.copy(fkb[:].rearrange("p (h w) -> p h w", w=128),
                           frecK[:].unsqueeze(2).broadcast_to((128, NH, 128)))
            fkr = fkb[:].rearrange("p (h w) -> p h w", w=128)
            for dc in range(2):
                nc.vector.tensor_tensor(
                    fr[dc][:, :, 128:256], fr[dc][:, :, 128:256], fkr, OP.mult)
            # one DMA transpose per dc: G = f^T, per head [Q1T 128 | K1T 128]
            G = [G_p.tile([128, 4096], BF16, tag=f"G{dc}", name=f"G{dc}")
                 for dc in range(2)]
            for dc in range(2):
                nc.sync.dma_start_transpose(
                    G[dc][:].rearrange("p (b l) -> p b l", l=128), f[dc][:])
            h.update(G=G, f=f, frecQ=frecQ)

        def prepB2(c):
            h = H[c]
            G, f, frecQ = h["G"], h["f"], h["frecQ"]
            # grams per head: psum [S1 128 | A 128 | kd 1 | dn 1]
            kd = cols_p.tile([128, NH], F32, tag="kd", name="kd")
            dna = cols_p.tile([128, NH], F32, tag="dna", name="dna")
            Shat = sh_p.tile([128, NH * 128], BF16, tag="sh", name="sh")
            NT = nt_p.tile([128, NH * 128], BF16, tag="nt", name="nt")

            pend = []

            def gram_front(hd):
                ps = psG_p.tile([128, 258], F32, tag="gg", name="gg")
                b = hd * 256
                for dc in range(2):
                    nc.tensor.matmul(ps[:, 0:256], G[dc][:, b + 128:b + 256],
                                     G[dc][:, b:b + 256],
                                     start=(dc == 0), stop=(dc == 1))
                sa = sa_p.tile([128, 256], BF16, tag="sa", name="sa")
                nc.scalar.copy(sa[:], ps[:, 0:256])
                nc.gpsimd.tensor_mul(Shat[:, hd * 128:(hd + 1) * 128],
                                     sa[:, 0:128], t_mUI[:])
                nc.gpsimd.tensor_mul(NT[:, hd * 128:(hd + 1) * 128],
                                     sa[:, 128:256], t_mUS[:])
                return ps

            def gram_back(hd, ps):
                b = hd * 256
                for dc in range(2):
                    nc.tensor.matmul(ps[:, 256:257], G[dc][:, b + 128:b + 256],
                                     t_rb[dc][:, hd:hd + 1],
                                     start=(dc == 0), stop=False)
                nc.tensor.matmul(ps[:, 256:257], NT[:, hd * 128:(hd + 1) * 128],
                                 t_ones[:], start=False, stop=True)
                for dc in range(2):
                    nc.tensor.matmul(ps[:, 257:258], G[dc][:, b:b + 128],
                                     t_rb[dc][:, hd:hd + 1],
                                     start=(dc == 0), stop=False)
                nc.tensor.matmul(ps[:, 257:258], Shat[:, hd * 128:(hd + 1) * 128],
                                 t_ones[:], start=False, stop=True)
                nc.vector.tensor_copy(kd[:, hd:hd + 1], ps[:, 256:257])
                nc.vector.tensor_copy(dna[:, hd:hd + 1], ps[:, 257:258])

            LAG = 2
            for hd in range(NH):
                pend.append((hd, gram_front(hd)))
                if hd >= LAG:
                    gram_back(*pend[hd - LAG])
            for i in range(NH - LAG, NH):
                gram_back(*pend[i])
            if c == 0:
                nc.vector.memset(kd[0:1, :], 1.0)

            # chunk-level columns
            ceps = cols_p.tile([128, NH], F32, tag="ceps", name="ceps")
            nc.vector.tensor_scalar_add(ceps[:], kd[:], EPS)
            cc = cols_p.tile([128, NH], F32, tag="cc", name="cc")
            nc.vector.reciprocal(cc[:], ceps[:])
            t0 = cols_p.tile([128, NH], F32, tag="t0", name="t0")
            nc.vector.tensor_mul(t0[:], kd[:], cc[:])
            cb = cols_p.tile([128, NH], F32, tag="cb", name="cb")
            nc.vector.tensor_mul(cb[:], t0[:], h["sig"][:])
            cbc = cols_p.tile([128, NH], F32, tag="cbc", name="cbc")
            nc.vector.tensor_mul(cbc[:], cb[:], cc[:])
            dnm = cols_p.tile([128, NH], F32, tag="dnm", name="dnm")
            nc.vector.tensor_mul(dnm[:], dna[:], frecQ[:])
            dne = cols_p.tile([128, NH], F32, tag="dne", name="dne")
            nc.vector.tensor_scalar_add(dne[:], dnm[:], EPS)
            dnr = cols_p.tile([128, NH], F32, tag="dnr", name="dnr")
            nc.vector.reciprocal(dnr[:], dne[:])
            dnr2 = cols_p.tile([128, NH], F32, tag="dnr2", name="dnr2")
            nc.vector.tensor_scalar_mul(dnr2[:], dnr[:], SCALE)
            dnrS = cols_p.tile([128, NH], F32, tag="dnrS", name="dnrS")
            nc.vector.tensor_mul(dnrS[:], dnr2[:], frecQ[:])
            # t1 = cb * V (cb broadcast materialized on Act for packed DVE mul)
            cbb = V_p.tile([128, 1024], BF16, tag="cbb", name="cbb")
            nc.scalar.copy(cbb[:].rearrange("p (h v) -> p h v", v=64),
                           cb[:].unsqueeze(2).broadcast_to((128, NH, 64)))
            t1 = t1_p.tile([128, 1024], BF16, tag="t1", name="t1")
            nc.vector.tensor_mul(t1[:], h["Vall"][:], cbb[:])
            h.update(G=G, f=f, Shat=Shat, NT=NT, cbc=cbc, dnrS=dnrS, t1=t1)

        def bscale(out, ps, colvec, g):
            """out (bf16) = psum [128,512] * per-head column broadcast."""
            nc.vector.tensor_tensor(
                out.rearrange("p (h v) -> p h v", v=64),
                ps[:].rearrange("p (h v) -> p h v", v=64),
                colvec[:, g * 8:(g + 1) * 8].unsqueeze(2).broadcast_to((128, 8, 64)),
                OP.mult)

        def scan(c):
            cs = slice(c * 128, (c + 1) * 128)
            h = H.pop(c)
            G, f, cbc, dnrS = h["G"], h["f"], h["cbc"], h["dnrS"]
            # ---- KW + X0 = t1 - cbc*KW
            X0 = x0_p.tile([128, 1024], BF16, tag="x0", name="x0")
            for g in range(2):
                ps = psS_p.tile([128, 512], F32, tag="ss", name="ss")
                for j in range(8):
                    hd = g * 8 + j
                    b = hd * 256
                    for dc in range(2):
                        nc.tensor.matmul(ps[:, j * 64:(j + 1) * 64],
                                         G[dc][:, b + 128:b + 256],
                                         t_Wb[:, hd * 128 + dc * 64:
                                              hd * 128 + (dc + 1) * 64],
                                         start=(dc == 0), stop=(dc == 1))
                mg = mg_p.tile([128, 512], BF16, tag="mg", name="mg")
                bscale(mg[:], ps, cbc, g)
                nc.vector.tensor_sub(X0[:, g * 512:(g + 1) * 512],
                                     h["t1"][:, g * 512:(g + 1) * 512], mg[:])
            # ---- Horner iterations: Z <- X0 - cbc*(NT^T Z); last writes t_Y
            y65 = t_Y[:].rearrange("p (h w) -> p h w", w=65)
            Zin = X0
            for it in range(NHORN):
                last = (it == NHORN - 1)
                Zout = None if last else z_p.tile([128, 1024], BF16,
                                                  tag=f"z{it}", name=f"z{it}")
                for g in range(2):
                    ps = psS_p.tile([128, 512], F32, tag="ss", name="ss")
                    for j in range(8):
                        hd = g * 8 + j
                        nc.tensor.matmul(ps[:, j * 64:(j + 1) * 64],
                                         h["NT"][:, hd * 128:(hd + 1) * 128],
                                         Zin[:, hd * 64:(hd + 1) * 64],
                                         start=True, stop=True)
                    mg = mg_p.tile([128, 512], BF16, tag="mg", name="mg")
                    bscale(mg[:], ps, cbc, g)
                    if last:
                        nc.vector.tensor_sub(
                            y65[:, g * 8:(g + 1) * 8, 0:64],
                            X0[:, g * 512:(g + 1) * 512].rearrange(
                                "p (h v) -> p h v", v=64),
                            mg[:].rearrange("p (h v) -> p h v", v=64))
                    else:
                        nc.vector.tensor_sub(Zout[:, g * 512:(g + 1) * 512],
                                             X0[:, g * 512:(g + 1) * 512], mg[:])
                Zin = Zout
            # ---- outc = (QW + tril_incl(S1) Y) * dnrS
            outc = oc_p.tile([128, 1024], BF16, tag="oc", name="oc")
            for g in range(2):
                ps = psS_p.tile([128, 512], F32, tag="ss", name="ss")
                for j in range(8):
                    hd = g * 8 + j
                    b = hd * 256
                    for dc in range(2):
                        nc.tensor.matmul(ps[:, j * 64:(j + 1) * 64],
                                         G[dc][:, b:b + 128],
                                         t_Wb[:, hd * 128 + dc * 64:
                                              hd * 128 + (dc + 1) * 64],
                                         start=(dc == 0), stop=False)
                    nc.tensor.matmul(ps[:, j * 64:(j + 1) * 64],
                                     h["Shat"][:, hd * 128:(hd + 1) * 128],
                                     t_Y[:, hd * 65:hd * 65 + 64],
                                     start=False, stop=True)
                bscale(outc[:, g * 512:(g + 1) * 512], ps, dnrS, g)
            # ---- transpose outc -> outT [i, t] via one DMA
            outT = oT_p.tile([128, 1024], BF16, tag="oT", name="oT")
            nc.sync.dma_start_transpose(
                outT[:].rearrange("p (b l) -> p b l", l=128), outc[:])
            # ---- W state update: Wm += K1^T [Y|1] (col 64 of each 65-block
            # gives the r increment); Wb = bf16(Wm)
            for rr in range(6):
                hds = list(range(3 * rr, min(3 * rr + 3, NH)))
                nh_r = len(hds)
                ps = psS_p.tile([128, nh_r * 130], F32, tag="ss", name="ss")
                for j, hd in enumerate(hds):
                    for dc in range(2):
                        nc.tensor.matmul(
                            ps[:, j * 130 + dc * 65:j * 130 + dc * 65 + 65],
                            f[dc][:, hd * 256 + 128:hd * 256 + 256],
                            t_Y[:, hd * 65:(hd + 1) * 65], start=True, stop=True)
                nc.vector.tensor_add(
                    t_Wm[:, hds[0] * 128:(hds[-1] + 1) * 128].rearrange(
                        "p (j d v) -> p j d v", d=2, v=64),
                    ps[:].rearrange("p (j d w) -> p j d w", d=2, w=65)[
                        :, :, :, 0:64],
                    t_Wm[:, hds[0] * 128:(hds[-1] + 1) * 128].rearrange(
                        "p (j d v) -> p j d v", d=2, v=64))
                for dc in range(2):
                    nc.vector.tensor_add(
                        t_r[dc][:, hds[0]:hds[-1] + 1],
                        ps[:].rearrange("p (j w) -> p j w", w=130)[
                            :, :, dc * 65 + 64:dc * 65 + 65].squeeze(2),
                        t_r[dc][:, hds[0]:hds[-1] + 1])
                nc.scalar.copy(t_Wb[:, hds[0] * 128:(hds[-1] + 1) * 128],
                               t_Wm[:, hds[0] * 128:(hds[-1] + 1) * 128])
            for dc in range(2):
                nc.scalar.copy(t_rb[dc][:], t_r[dc][:])
            # ---- output projection + residual + LN
            x = x_p.tile([128, DM], BF16, tag="x", name="x")
            for og in range(2):
                ps = psP_p.tile([128, 512], F32, tag="pp", name="pp")
                for ic in range(8):
                    nc.tensor.matmul(ps[:], outT[:, ic * 128:(ic + 1) * 128],
                                     t_wo[ic][:, og * 512:(og + 1) * 512],
                                     start=(ic == 0), stop=(ic == 7))
                nc.vector.tensor_add(x[:, og * 512:(og + 1) * 512], ps[:],
                                     h["hr"][:, og * 512:(og + 1) * 512])
            xsum = cols_p.tile([128, 1], F32, tag="xsum", name="xsum")
            nc.vector.tensor_reduce(xsum[:], x[:], AX.X, OP.add)
            nmu = cols_p.tile([128, 1], F32, tag="nmu", name="nmu")
            nc.vector.tensor_scalar_mul(nmu[:], xsum[:], -1.0 / DM)
            nc.vector.tensor_scalar_add(x[:], x[:], nmu[:])
            vscr = oc_p.tile([128, DM], BF16, tag="oc", name="vscr")
            var = cols_p.tile([128, 1], F32, tag="var", name="var")
            nc.vector.scalar_tensor_tensor(vscr[:], x[:], 1.0, x[:],
                                           OP.mult, OP.mult, accum_out=var[:])
            vare = cols_p.tile([128, 1], F32, tag="vare", name="vare")
            nc.vector.tensor_scalar(vare[:], var[:], 1.0 / DM, float(LN_EPS),
                                    OP.mult, OP.add)
            sd = cols_p.tile([128, 1], F32, tag="sd", name="sd")
            nc.scalar.sqrt(sd[:], vare[:])
            rstd = cols_p.tile([128, 1], F32, tag="rstd", name="rstd")
            nc.vector.reciprocal(rstd[:], sd[:])
            nc.vector.scalar_tensor_tensor(x[:], x[:], rstd[:], t_lng[:],
                                           OP.mult, OP.mult)
            nc.vector.tensor_add(x[:], x[:], t_lnb[:])
            nc.sync.dma_start(d_out[cs, :], x[:])

        order = os.environ.get("KORDER", "ii")
        prepA(0)
        prepB1(0)
        prepB2(0)
        for c in range(n_chunks):
            if c + 1 < n_chunks:
                prepA(c + 1)
                if order == "i":
                    prepB1(c + 1)
            scan(c)
            if c + 1 < n_chunks:
                if order != "i":
                    prepB1(c + 1)
                prepB2(c + 1)

    return nc


# ---------------------------------------------------------------- host side
def _prep_core_inputs(h_b, W_qkvb, W_o, ln_g, ln_b):
    bf16 = ml_dtypes.bfloat16
    hT = np.ascontiguousarray(h_b.T).astype(bf16)                  # [1024, 2048]
    Wr = W_qkvb.reshape(NH, 193, DM)
    wq = np.empty((DM, 3088), dtype=bf16)
    wq[:, 0:1024] = Wr[:, 64:128, :].reshape(1024, DM).T           # K
    wq[:, 1024:2048] = Wr[:, 0:64, :].reshape(1024, DM).T          # Q
    wq[:, 2048:3072] = Wr[:, 128:192, :].reshape(1024, DM).T       # V
    wq[:, 3072:3088] = Wr[:, 192, :].T                             # b
    woT = np.ascontiguousarray(W_o.T).astype(bf16)                 # [i, o]
    lng = np.broadcast_to(ln_g[None, :], (128, DM)).astype(bf16).copy()
    lnb = np.broadcast_to(ln_b[None, :], (128, DM)).astype(bf16).copy()
    ii, jj = np.indices((128, 128))
    mUI = (jj >= ii).astype(bf16)
    mUS = (jj > ii).astype(bf16)
    return {"hT": hT, "hres": h_b.astype(bf16),
            "wqkv": wq, "woT": woT, "lng": lng, "lnb": lnb,
            "maskUI": mUI, "maskUS": mUS}


_cached = {}


def kernel(h, W_qkvb, W_o, ln_g, ln_b):
    h = np.asarray(h, np.float32)
    W_qkvb = np.asarray(W_qkvb, np.float32)
    W_o = np.asarray(W_o, np.float32)
    ln_g = np.asarray(ln_g, np.float32)
    ln_b = np.asarray(ln_b, np.float32)
    if "nc" not in _cached:
        _cached["nc"] = build_program()
    nc = _cached["nc"]
    in_maps = [_prep_core_inputs(h[:, b, :], W_qkvb, W_o, ln_g, ln_b)
               for b in range(BSZ)]
    res = run_bass_kernel_spmd(nc, in_maps, list(range(BSZ)),
                               trace=os.environ.get("BASS_TRACE", "") == "1")
    out = np.stack([res.results[b]["out"] for b in range(BSZ)], axis=1)
    kernel.last_exec_time_ns = res.exec_time_ns
    return out.astype(np.float32)


# revision 47
# speedup vs baseline: 106.6468x; 1.0496x over previous
"""Trainium2 Bass kernel for the CudaNorm FastWeight DPFP transformer layer.

Sharding: batch (8) across the 8 cores; each core runs its batch's full layer.

v3: head-batched DVE/Act ops via 3D strided APs, [K|Q|V|b] projection layout,
interleaved per-head [Q|K] feature tiles so one SBUF->SBUF DMA transpose per
dc produces the whole feature-major G tile, kd/dn via PE column-sum matmuls
accumulated onto K.r/Q.r psum groups, unnormalized-Q trick, depth-1 Horner
Neumann solve with batched cbc post-scaling, stage-major scan with 8-head
psum packing, prepA/scan/prepB software pipelining across chunks.
"""
import os
import numpy as np
import ml_dtypes

import concourse.bass as bass
import concourse.mybir as mybir
from concourse.bass_utils import run_bass_kernel_spmd
from concourse.tile import TileContext
from concourse.vector_clock import ScopedClock, VectorClock
from contextlib import ExitStack

F32 = mybir.dt.float32
BF16 = mybir.dt.bfloat16
AF = mybir.ActivationFunctionType
OP = mybir.AluOpType
AX = mybir.AxisListType

SLEN, BSZ, DM = 2048, 8, 1024
NH, DH, NROLL = 16, 64, 2
D = 2 * NROLL * DH            # 256 feature dim
C = 128                       # chunk length
NCH = SLEN // C               # 16 chunks
EPS, LN_EPS = 1e-5, 1e-5
SCALE = 1.0 / float(np.sqrt(DH))
NHORN = 1                     # Horner/Neumann solve iterations

# ---------------------------------------------------------------- tile ctx
MAXW = 2


class PatchedTileContext(TileContext):
    """Work around walrus TPB sync-command limits: each instruction carries at
    most 2 sync commands (waits+updates); hoist excess waits onto preceding
    same-engine NoOps (1 wait each), and emit the kernel-tail drain's waits
    one-per-nop on SP."""

    def _lower_ordered_insts(self, ordered):
        for bb_name in list(ordered.keys()):
            new = []
            for inst in ordered[bb_name]:
                si = inst.sync_info
                nupd = len(si.on_update) if si is not None and si.on_update else 0
                maxw = max(0, MAXW - nupd)
                if si is not None and si.on_wait and len(si.on_wait) > maxw:
                    waits = list(si.on_wait)
                    excess = waits if maxw == 0 else waits[:-maxw]
                    keep = [] if maxw == 0 else waits[-maxw:]
                    for w in excess:
                        nop = mybir.InstNoOp(
                            name=self.nc.get_next_instruction_name(),
                            engine=inst.engine, ins=[], outs=[])
                        nop.sync_info = mybir.SyncInfo(on_wait=[w], on_update=[])
                        new.append(nop)
                    inst.sync_info = mybir.SyncInfo(
                        on_wait=keep, on_update=list(si.on_update or []))
                new.append(inst)
            ordered[bb_name] = new
        return super()._lower_ordered_insts(ordered)

    def _drain_and_barrier(self, tick_clock, wait_clock):
        gc = tick_clock.global_clock
        n = len(gc)
        for p in range(n):
            if gc[p] > 0:
                vc = VectorClock([gc[i] if i == p else 0 for i in range(n)])
                nop = self.nc.sync.nop(nofuse=True)
                wait_clock.add_sem_waits(nop.ins, ScopedClock({None: vc}))
        self.nc.sync.drain()
        self.nc.all_engine_barrier()
        assert self.sems is not None
        popped = self.nc._tile_sem_poison_stack.pop()
        assert popped is self._sem_poison
        self.nc.clear_and_free_semaphores(list(self.sems.allocated().values()))
        self.nc.all_engine_barrier()


# ---------------------------------------------------------------- program
def build_program(n_chunks=NCH):
    nc = bass.Bass()
    d_hT = nc.declare_dram_parameter("hT", [DM, SLEN], BF16, isOutput=False)
    d_hres = nc.declare_dram_parameter("hres", [SLEN, DM], BF16, isOutput=False)
    d_w = nc.declare_dram_parameter("wqkv", [DM, 3088], BF16, isOutput=False)
    d_wo = nc.declare_dram_parameter("woT", [DM, DM], BF16, isOutput=False)
    d_lng = nc.declare_dram_parameter("lng", [128, DM], BF16, isOutput=False)
    d_lnb = nc.declare_dram_parameter("lnb", [128, DM], BF16, isOutput=False)
    d_mUI = nc.declare_dram_parameter("maskUI", [128, 128], BF16, isOutput=False)
    d_mUS = nc.declare_dram_parameter("maskUS", [128, 128], BF16, isOutput=False)
    d_out = nc.declare_dram_parameter("out", [SLEN, DM], BF16, isOutput=True)

    with PatchedTileContext(nc) as tc, ExitStack() as ctx:
        P = lambda name, bufs, **kw: ctx.enter_context(
            tc.tile_pool(name=name, bufs=bufs, **kw))
        const = P("const", 1)
        state = P("state", 1)
        hts_p = P("hts", 2)
        xp_p = P("xp", 1)
        f_p = P("f", 2)
        G_p = P("G", 2)
        V_p = P("V", 1)
        t1_p = P("t1", 2)
        cols_p = P("cols", 2)
        sa_p = P("sa", 2)
        sh_p = P("sh", 2)
        nt_p = P("nt", 2)
        z_p = P("z", 1)
        mg_p = P("mg", 2)
        oc_p = P("oc", 1)
        oT_p = P("oT", 1)
        hr_p = P("hr", 2)
        x_p = P("x", 2)
        psP_p = P("psP", 2, space="PSUM")
        psG_p = P("psG", 3, space="PSUM")
        psS_p = P("psS", 3, space="PSUM")

        # ---- constants
        t_mUI = const.tile([128, 128], BF16, tag="mUI", name="mUI")
        t_mUS = const.tile([128, 128], BF16, tag="mUS", name="mUS")
        t_lng = const.tile([128, DM], BF16, tag="lng", name="lng")
        t_lnb = const.tile([128, DM], BF16, tag="lnb", name="lnb")
        t_ones = const.tile([128, 1], BF16, tag="ones", name="ones")
        nc.vector.memset(t_ones[:], 1.0)
        t_w = [const.tile([128, 3088], BF16, tag=f"w{mc}", name=f"w{mc}")
               for mc in range(8)]
        t_wo = [const.tile([128, DM], BF16, tag=f"wo{ic}", name=f"wo{ic}")
                for ic in range(8)]

        def load_consts():
            for mc in range(8):
                nc.sync.dma_start(t_w[mc][:], d_w[mc * 128:(mc + 1) * 128, :])
            nc.sync.dma_start(t_mUI[:], d_mUI[:])
            nc.sync.dma_start(t_mUS[:], d_mUS[:])
            for ic in range(8):
                nc.sync.dma_start(t_wo[ic][:], d_wo[ic * 128:(ic + 1) * 128, :])
            nc.sync.dma_start(t_lng[:], d_lng[:])
            nc.sync.dma_start(t_lnb[:], d_lnb[:])

        # ---- state
        t_Wm = state.tile([128, NH * 128], F32, tag="Wm", name="Wm")
        nc.vector.memset(t_Wm[:], 0.0)
        t_Wb = state.tile([128, NH * 128], BF16, tag="Wb", name="Wb")
        nc.vector.memset(t_Wb[:], 0.0)
        t_r, t_rb = [], []
        for dc in range(2):
            r = state.tile([128, NH], F32, tag=f"r{dc}", name=f"r{dc}")
            nc.vector.memset(r[:], 0.0)
            t_r.append(r)
            rb = state.tile([128, NH], BF16, tag=f"rb{dc}", name=f"rb{dc}")
            nc.vector.memset(rb[:], 0.0)
            t_rb.append(rb)
        # Y with per-head stride 65; col 64 stays 1.0 so the pW matmuls also
        # produce the per-chunk r increment (sum_t K[t,f]) for free.
        t_Y = state.tile([128, NH * 65], BF16, tag="Y65", name="Y65")
        nc.vector.memset(t_Y[:], 1.0)

        H = {}

        def prepA_dma(c):
            cs = slice(c * 128, (c + 1) * 128)
            hts = hts_p.tile([128, 1024], BF16, tag="hts", name="hts")
            nc.sync.dma_start(
                hts[:].rearrange("p (mc t) -> p mc t", t=128),
                d_hT[:, cs].rearrange("(mc p) t -> p mc t", p=128))
            hr = hr_p.tile([128, DM], BF16, tag="hr", name="hr")
            nc.sync.dma_start(hr[:], d_hres[cs, :])
            return hts, hr

        def prepA(c, pre=None):
            hts, hr = pre if pre is not None else prepA_dma(c)
            # xpC: per head [relu(Q) 64 | relu(-Q) 64 | relu(K) 64 | relu(-K) 64]
            xpC = xp_p.tile([128, 4096], BF16, tag="xpC", name="xpC")
            xr = xpC[:].rearrange("p (h w) -> p h w", w=256)
            Vall = V_p.tile([128, 1024], BF16, tag="V", name="V")
            sig = cols_p.tile([128, NH], F32, tag="sig", name="sig")
            for og in range(6):
                ps = psP_p.tile([128, 512], F32, tag="pp", name="pp")
                for mc in range(8):
                    nc.tensor.matmul(ps[:], hts[:, mc * 128:(mc + 1) * 128],
                                     t_w[mc][:, og * 512:(og + 1) * 512],
                                     start=(mc == 0), stop=(mc == 7))
                psr = ps[:].rearrange("p (h v) -> p h v", v=64)
                if og < 4:      # K (og 0,1) / Q (og 2,3), heads 8*(og%2)..
                    o = 128 if og < 2 else 0
                    hs = slice((og % 2) * 8, (og % 2) * 8 + 8)
                    nc.scalar.activation(xr[:, hs, o:o + 64], psr, AF.Relu)
                    nc.scalar.activation(xr[:, hs, o + 64:o + 128], psr,
                                         AF.Relu, scale=-1.0)
                else:           # V
                    gv = og - 4
                    nc.scalar.copy(Vall[:, gv * 512:(gv + 1) * 512], ps[:])
            psb = psP_p.tile([128, NH], F32, tag="pp", name="pp")
            for mc in range(8):
                nc.tensor.matmul(psb[:], hts[:, mc * 128:(mc + 1) * 128],
                                 t_w[mc][:, 3072:3088],
                                 start=(mc == 0), stop=(mc == 7))
            nc.scalar.activation(sig[:], psb[:], AF.Sigmoid)
            H[c] = dict(xpC=xpC, Vall=Vall, sig=sig, hr=hr)

        def prepB1(c):
            h = H[c]
            # f tiles: per dc, per head [Q-roll-dc 128 | K-roll-dc 128]
            f = [f_p.tile([128, 4096], BF16, tag=f"f{dc}", name=f"f{dc}")
                 for dc in range(2)]
            fr = [f[dc][:].rearrange("p (h w) -> p h w", w=256)
                  for dc in range(2)]
            xpC = h["xpC"]
            xr = xpC[:].rearrange("p (h w) -> p h w", w=256)
            for dc in range(2):
                rl = dc + 1
                for o in (0, 128):  # Q block, K block per head
                    nc.vector.tensor_mul(fr[dc][:, :, o + rl:o + 128],
                                         xr[:, :, o + rl:o + 128],
                                         xr[:, :, o:o + 128 - rl])
                    nc.vector.tensor_mul(fr[dc][:, :, o:o + rl],
                                         xr[:, :, o:o + rl],
                                         xr[:, :, o + 128 - rl:o + 128])
            # feature sums: packed-2x bf16 pair-adds fold 256 values down to
            # 64 per head, then one small fp32 reduce
            frec = []
            for side in range(2):
                o = side * 128
                b1 = V_p.tile([128, 1024], BF16, tag="b1", name="b1")
                b1r = b1[:].rearrange("p (h w) -> p h w", w=64)
                nc.vector.tensor_add(b1r, fr[0][:, :, o:o + 64],
                                     fr[0][:, :, o + 64:o + 128])
                nc.vector.tensor_add(b1r, fr[1][:, :, o:o + 64], b1r)
                nc.vector.tensor_add(b1r, fr[1][:, :, o + 64:o + 128], b1r)
                fs = cols_p.tile([128, NH], F32, tag=f"fs{side}", name=f"fs{side}")
                nc.vector.tensor_reduce(fs[:], b1r, AX.X, OP.add)
                fc = cols_p.tile([128, NH], F32, tag=f"fc{side}", name=f"fc{side}")
                nc.vector.reciprocal(fc[:], fs[:])
                frec.append(fc)
            frecQ, frecK = frec[0], frec[1]
            # materialize the frecK broadcast on Act so the DVE muls run packed
            fkb = V_p.tile([128, NH * 128], BF16, tag="fkb", name="fkb")
            nc.scalar.copy(fkb[:].rearrange("p (h w) -> p h w", w=128),
                           frecK[:].unsqueeze(2).broadcast_to((128, NH, 128)))
            fkr = fkb[:].rearrange("p (h w) -> p h w", w=128)
            for dc in range(2):
                nc.vector.tensor_tensor(
                    fr[dc][:, :, 128:256], fr[dc][:, :, 128:256], fkr, OP.mult)
            # one DMA transpose per dc: G = f^T, per head [Q1T 128 | K1T 128]
            G = [G_p.tile([128, 4096], BF16, tag=f"G{dc}", name=f"G{dc}")
                 for dc in range(2)]
            for dc in range(2):
                nc.sync.dma_start_transpose(
                    G[dc][:].rearrange("p (b l) -> p b l", l=128), f[dc][:])
            h.update(G=G, f=f, frecQ=frecQ)

        def prepB2(c):
            h = H[c]
            G, f, frecQ = h["G"], h["f"], h["frecQ"]
            # grams per head: psum [S1 128 | A 128 | kd 1 | dn 1]
            kd = cols_p.tile([128, NH], F32, tag="kd", name="kd")
            dna = cols_p.tile([128, NH], F32, tag="dna", name="dna")
            Shat = sh_p.tile([128, NH * 128], BF16, tag="sh", name="sh")
            NT = nt_p.tile([128, NH * 128], BF16, tag="nt", name="nt")

            pend = []

            def gram_front(hd):
                ps = psG_p.tile([128, 258], F32, tag="gg", name="gg")
                b = hd * 256
                for dc in range(2):
                    nc.tensor.matmul(ps[:, 0:256], G[dc][:, b + 128:b + 256],
                                     G[dc][:, b:b + 256],
                                     start=(dc == 0), stop=(dc == 1))
                sa = sa_p.tile([128, 256], BF16, tag="sa", name="sa")
                nc.scalar.copy(sa[:], ps[:, 0:256])
                nc.gpsimd.tensor_mul(Shat[:, hd * 128:(hd + 1) * 128],
                                     sa[:, 0:128], t_mUI[:])
                nc.gpsimd.tensor_mul(NT[:, hd * 128:(hd + 1) * 128],
                                     sa[:, 128:256], t_mUS[:])
                return ps

            def gram_back(hd, ps):
                b = hd * 256
                for dc in range(2):
                    nc.tensor.matmul(ps[:, 256:257], G[dc][:, b + 128:b + 256],
                                     t_rb[dc][:, hd:hd + 1],
                                     start=(dc == 0), stop=False)
                nc.tensor.matmul(ps[:, 256:257], NT[:, hd * 128:(hd + 1) * 128],
                                 t_ones[:], start=False, stop=True)
                for dc in range(2):
                    nc.tensor.matmul(ps[:, 257:258], G[dc][:, b:b + 128],
                                     t_rb[dc][:, hd:hd + 1],
                                     start=(dc == 0), stop=False)
                nc.tensor.matmul(ps[:, 257:258], Shat[:, hd * 128:(hd + 1) * 128],
                                 t_ones[:], start=False, stop=True)
                nc.vector.tensor_copy(kd[:, hd:hd + 1], ps[:, 256:257])
                nc.vector.tensor_copy(dna[:, hd:hd + 1], ps[:, 257:258])

            LAG = 2
            for hd in range(NH):
                pend.append((hd, gram_front(hd)))
                if hd >= LAG:
                    gram_back(*pend[hd - LAG])
            for i in range(NH - LAG, NH):
                gram_back(*pend[i])
            if c == 0:
                nc.vector.memset(kd[0:1, :], 1.0)

            # chunk-level columns
            ceps = cols_p.tile([128, NH], F32, tag="ceps", name="ceps")
            nc.vector.tensor_scalar_add(ceps[:], kd[:], EPS)
            cc = cols_p.tile([128, NH], F32, tag="cc", name="cc")
            nc.vector.reciprocal(cc[:], ceps[:])
            t0 = cols_p.tile([128, NH], F32, tag="t0", name="t0")
            nc.vector.tensor_mul(t0[:], kd[:], cc[:])
            cb = cols_p.tile([128, NH], F32, tag="cb", name="cb")
            nc.vector.tensor_mul(cb[:], t0[:], h["sig"][:])
            cbc = cols_p.tile([128, NH], F32, tag="cbc", name="cbc")
            nc.vector.tensor_mul(cbc[:], cb[:], cc[:])
            dnm = cols_p.tile([128, NH], F32, tag="dnm", name="dnm")
            nc.vector.tensor_mul(dnm[:], dna[:], frecQ[:])
            dne = cols_p.tile([128, NH], F32, tag="dne", name="dne")
            nc.vector.tensor_scalar_add(dne[:], dnm[:], EPS)
            dnr = cols_p.tile([128, NH], F32, tag="dnr", name="dnr")
            nc.vector.reciprocal(dnr[:], dne[:])
            dnr2 = cols_p.tile([128, NH], F32, tag="dnr2", name="dnr2")
            nc.vector.tensor_scalar_mul(dnr2[:], dnr[:], SCALE)
            dnrS = cols_p.tile([128, NH], F32, tag="dnrS", name="dnrS")
            nc.vector.tensor_mul(dnrS[:], dnr2[:], frecQ[:])
            # t1 = cb * V (cb broadcast materialized on Act for packed DVE mul)
            cbb = V_p.tile([128, 1024], BF16, tag="cbb", name="cbb")
            nc.scalar.copy(cbb[:].rearrange("p (h v) -> p h v", v=64),
                           cb[:].unsqueeze(2).broadcast_to((128, NH, 64)))
            t1 = t1_p.tile([128, 1024], BF16, tag="t1", name="t1")
            nc.vector.tensor_mul(t1[:], h["Vall"][:], cbb[:])
            h.update(G=G, f=f, Shat=Shat, NT=NT, cbc=cbc, dnrS=dnrS, t1=t1)

        def bscale(out, ps, colvec, g):
            """out (bf16) = psum [128,512] * per-head column broadcast."""
            nc.vector.tensor_tensor(
                out.rearrange("p (h v) -> p h v", v=64),
                ps[:].rearrange("p (h v) -> p h v", v=64),
                colvec[:, g * 8:(g + 1) * 8].unsqueeze(2).broadcast_to((128, 8, 64)),
                OP.mult)

        def scan(c):
            cs = slice(c * 128, (c + 1) * 128)
            h = H.pop(c)
            G, f, cbc, dnrS = h["G"], h["f"], h["cbc"], h["dnrS"]
            # ---- KW + X0 = t1 - cbc*KW (in place over t1)
            X0 = h["t1"]
            for g in range(2):
                ps = psS_p.tile([128, 512], F32, tag="ss", name="ss")
                for j in range(8):
                    hd = g * 8 + j
                    b = hd * 256
                    for dc in range(2):
                        nc.tensor.matmul(ps[:, j * 64:(j + 1) * 64],
                                         G[dc][:, b + 128:b + 256],
                                         t_Wb[:, hd * 128 + dc * 64:
                                              hd * 128 + (dc + 1) * 64],
                                         start=(dc == 0), stop=(dc == 1))
                mg = mg_p.tile([128, 512], BF16, tag="mg", name="mg")
                bscale(mg[:], ps, cbc, g)
                nc.vector.tensor_sub(X0[:, g * 512:(g + 1) * 512],
                                     X0[:, g * 512:(g + 1) * 512], mg[:])
            # ---- Horner iterations: Z <- X0 - cbc*(NT^T Z); last writes t_Y
            y65 = t_Y[:].rearrange("p (h w) -> p h w", w=65)
            Zin = X0
            for it in range(NHORN):
                last = (it == NHORN - 1)
                Zout = None if last else z_p.tile([128, 1024], BF16,
                                                  tag=f"z{it}", name=f"z{it}")
                for g in range(2):
                    ps = psS_p.tile([128, 512], F32, tag="ss", name="ss")
                    for j in range(8):
                        hd = g * 8 + j
                        nc.tensor.matmul(ps[:, j * 64:(j + 1) * 64],
                                         h["NT"][:, hd * 128:(hd + 1) * 128],
                                         Zin[:, hd * 64:(hd + 1) * 64],
                                         start=True, stop=True)
                    mg = mg_p.tile([128, 512], BF16, tag="mg", name="mg")
                    bscale(mg[:], ps, cbc, g)
                    if last:
                        nc.vector.tensor_sub(
                            y65[:, g * 8:(g + 1) * 8, 0:64],
                            X0[:, g * 512:(g + 1) * 512].rearrange(
                                "p (h v) -> p h v", v=64),
                            mg[:].rearrange("p (h v) -> p h v", v=64))
                    else:
                        nc.vector.tensor_sub(Zout[:, g * 512:(g + 1) * 512],
                                             X0[:, g * 512:(g + 1) * 512], mg[:])
                Zin = Zout
            # ---- outc = (QW + tril_incl(S1) Y) * dnrS
            outc = oc_p.tile([128, 1024], BF16, tag="oc", name="oc")
            for g in range(2):
                ps = psS_p.tile([128, 512], F32, tag="ss", name="ss")
                for j in range(8):
                    hd = g * 8 + j
                    b = hd * 256
                    for dc in range(2):
                        nc.tensor.matmul(ps[:, j * 64:(j + 1) * 64],
                                         G[dc][:, b:b + 128],
                                         t_Wb[:, hd * 128 + dc * 64:
                                              hd * 128 + (dc + 1) * 64],
                                         start=(dc == 0), stop=False)
                    nc.tensor.matmul(ps[:, j * 64:(j + 1) * 64],
                                     h["Shat"][:, hd * 128:(hd + 1) * 128],
                                     t_Y[:, hd * 65:hd * 65 + 64],
                                     start=False, stop=True)
                bscale(outc[:, g * 512:(g + 1) * 512], ps, dnrS, g)
            # ---- transpose outc -> outT [i, t] via one DMA
            outT = oT_p.tile([128, 1024], BF16, tag="oT", name="oT")
            nc.sync.dma_start_transpose(
                outT[:].rearrange("p (b l) -> p b l", l=128), outc[:])
            # ---- W state update: Wm += K1^T [Y|1] (col 64 of each 65-block
            # gives the r increment); Wb = bf16(Wm)
            for rr in range(6):
                hds = list(range(3 * rr, min(3 * rr + 3, NH)))
                nh_r = len(hds)
                ps = psS_p.tile([128, nh_r * 130], F32, tag="ss", name="ss")
                for j, hd in enumerate(hds):
                    for dc in range(2):
                        nc.tensor.matmul(
                            ps[:, j * 130 + dc * 65:j * 130 + dc * 65 + 65],
                            f[dc][:, hd * 256 + 128:hd * 256 + 256],
                            t_Y[:, hd * 65:(hd + 1) * 65], start=True, stop=True)
                nc.vector.tensor_add(
                    t_Wm[:, hds[0] * 128:(hds[-1] + 1) * 128].rearrange(
                        "p (j d v) -> p j d v", d=2, v=64),
                    ps[:].rearrange("p (j d w) -> p j d w", d=2, w=65)[
                        :, :, :, 0:64],
                    t_Wm[:, hds[0] * 128:(hds[-1] + 1) * 128].rearrange(
                        "p (j d v) -> p j d v", d=2, v=64))
                for dc in range(2):
                    nc.vector.tensor_add(
                        t_r[dc][:, hds[0]:hds[-1] + 1],
                        ps[:].rearrange("p (j w) -> p j w", w=130)[
                            :, :, dc * 65 + 64:dc * 65 + 65].squeeze(2),
                        t_r[dc][:, hds[0]:hds[-1] + 1])
                nc.scalar.copy(t_Wb[:, hds[0] * 128:(hds[-1] + 1) * 128],
                               t_Wm[:, hds[0] * 128:(hds[-1] + 1) * 128])
            for dc in range(2):
                nc.scalar.copy(t_rb[dc][:], t_r[dc][:])
            # ---- output projection + residual + LN
            x = x_p.tile([128, DM], BF16, tag="x", name="x")
            for og in range(2):
                ps = psP_p.tile([128, 512], F32, tag="pp", name="pp")
                for ic in range(8):
                    nc.tensor.matmul(ps[:], outT[:, ic * 128:(ic + 1) * 128],
                                     t_wo[ic][:, og * 512:(og + 1) * 512],
                                     start=(ic == 0), stop=(ic == 7))
                nc.vector.tensor_add(x[:, og * 512:(og + 1) * 512], ps[:],
                                     h["hr"][:, og * 512:(og + 1) * 512])
            xsum = cols_p.tile([128, 1], F32, tag="xsum", name="xsum")
            nc.vector.tensor_reduce(xsum[:], x[:], AX.X, OP.add)
            nmu = cols_p.tile([128, 1], F32, tag="nmu", name="nmu")
            nc.vector.tensor_scalar_mul(nmu[:], xsum[:], -1.0 / DM)
            nc.vector.tensor_scalar_add(x[:], x[:], nmu[:])
            vscr = oc_p.tile([128, DM], BF16, tag="oc", name="vscr")
            var = cols_p.tile([128, 1], F32, tag="var", name="var")
            nc.vector.scalar_tensor_tensor(vscr[:], x[:], 1.0, x[:],
                                           OP.mult, OP.mult, accum_out=var[:])
            vare = cols_p.tile([128, 1], F32, tag="vare", name="vare")
            nc.vector.tensor_scalar(vare[:], var[:], 1.0 / DM, float(LN_EPS),
                                    OP.mult, OP.add)
            sd = cols_p.tile([128, 1], F32, tag="sd", name="sd")
            nc.scalar.sqrt(sd[:], vare[:])
            rstd = cols_p.tile([128, 1], F32, tag="rstd", name="rstd")
            nc.vector.reciprocal(rstd[:], sd[:])
            nc.vector.scalar_tensor_tensor(x[:], x[:], rstd[:], t_lng[:],
                                           OP.mult, OP.mult)
            nc.vector.tensor_add(x[:], x[:], t_lnb[:])
            nc.sync.dma_start(d_out[cs, :], x[:])

        order = os.environ.get("KORDER", "ii")
        pre0 = prepA_dma(0)
        load_consts()
        prepA(0, pre0)
        prepB1(0)
        prepB2(0)
        for c in range(n_chunks):
            if c + 1 < n_chunks:
                prepA(c + 1)
                if order == "i":
                    prepB1(c + 1)
            scan(c)
            if c + 1 < n_chunks:
                if order != "i":
                    prepB1(c + 1)
                prepB2(c + 1)

    return nc


# ---------------------------------------------------------------- host side
def _prep_core_inputs(h_b, W_qkvb, W_o, ln_g, ln_b):
    bf16 = ml_dtypes.bfloat16
    hT = np.ascontiguousarray(h_b.T).astype(bf16)                  # [1024, 2048]
    Wr = W_qkvb.reshape(NH, 193, DM)
    wq = np.empty((DM, 3088), dtype=bf16)
    wq[:, 0:1024] = Wr[:, 64:128, :].reshape(1024, DM).T           # K
    wq[:, 1024:2048] = Wr[:, 0:64, :].reshape(1024, DM).T          # Q
    wq[:, 2048:3072] = Wr[:, 128:192, :].reshape(1024, DM).T       # V
    wq[:, 3072:3088] = Wr[:, 192, :].T                             # b
    woT = np.ascontiguousarray(W_o.T).astype(bf16)                 # [i, o]
    lng = np.broadcast_to(ln_g[None, :], (128, DM)).astype(bf16).copy()
    lnb = np.broadcast_to(ln_b[None, :], (128, DM)).astype(bf16).copy()
    ii, jj = np.indices((128, 128))
    mUI = (jj >= ii).astype(bf16)
    mUS = (jj > ii).astype(bf16)
    return {"hT": hT, "hres": h_b.astype(bf16),
            "wqkv": wq, "woT": woT, "lng": lng, "lnb": lnb,
            "maskUI": mUI, "maskUS": mUS}


_cached = {}


def kernel(h, W_qkvb, W_o, ln_g, ln_b):
    h = np.asarray(h, np.float32)
    W_qkvb = np.asarray(W_qkvb, np.float32)
    W_o = np.asarray(W_o, np.float32)
    ln_g = np.asarray(ln_g, np.float32)
    ln_b = np.asarray(ln_b, np.float32)
    if "nc" not in _cached:
        _cached["nc"] = build_program()
    nc = _cached["nc"]
    in_maps = [_prep_core_inputs(h[:, b, :], W_qkvb, W_o, ln_g, ln_b)
               for b in range(BSZ)]
    res = run_bass_kernel_spmd(nc, in_maps, list(range(BSZ)),
                               trace=os.environ.get("BASS_TRACE", "") == "1")
    out = np.stack([res.results[b]["out"] for b in range(BSZ)], axis=1)
    kernel.last_exec_time_ns = res.exec_time_ns
    return out.astype(np.float32)


# revision 52
# speedup vs baseline: 106.7192x; 1.0007x over previous
"""Trainium2 Bass kernel for the CudaNorm FastWeight DPFP transformer layer.

Sharding: batch (8) across the 8 cores; each core runs its batch's full layer.

v3: head-batched DVE/Act ops via 3D strided APs, [K|Q|V|b] projection layout,
interleaved per-head [Q|K] feature tiles so one SBUF->SBUF DMA transpose per
dc produces the whole feature-major G tile, kd/dn via PE column-sum matmuls
accumulated onto K.r/Q.r psum groups, unnormalized-Q trick, depth-1 Horner
Neumann solve with batched cbc post-scaling, stage-major scan with 8-head
psum packing, prepA/scan/prepB software pipelining across chunks.
"""
import os
import numpy as np
import ml_dtypes

import concourse.bass as bass
import concourse.mybir as mybir
from concourse.bass_utils import run_bass_kernel_spmd
from concourse.tile import TileContext
from concourse.vector_clock import ScopedClock, VectorClock
from contextlib import ExitStack

F32 = mybir.dt.float32
BF16 = mybir.dt.bfloat16
AF = mybir.ActivationFunctionType
OP = mybir.AluOpType
AX = mybir.AxisListType

SLEN, BSZ, DM = 2048, 8, 1024
NH, DH, NROLL = 16, 64, 2
D = 2 * NROLL * DH            # 256 feature dim
C = 128                       # chunk length
NCH = SLEN // C               # 16 chunks
EPS, LN_EPS = 1e-5, 1e-5
SCALE = 1.0 / float(np.sqrt(DH))
NHORN = 1                     # Horner/Neumann solve iterations

# ---------------------------------------------------------------- tile ctx
MAXW = 2


class PatchedTileContext(TileContext):
    """Work around walrus TPB sync-command limits: each instruction carries at
    most 2 sync commands (waits+updates); hoist excess waits onto preceding
    same-engine NoOps (1 wait each), and emit the kernel-tail drain's waits
    one-per-nop on SP."""

    def _lower_ordered_insts(self, ordered):
        for bb_name in list(ordered.keys()):
            new = []
            for inst in ordered[bb_name]:
                si = inst.sync_info
                nupd = len(si.on_update) if si is not None and si.on_update else 0
                maxw = max(0, MAXW - nupd)
                if si is not None and si.on_wait and len(si.on_wait) > maxw:
                    waits = list(si.on_wait)
                    excess = waits if maxw == 0 else waits[:-maxw]
                    keep = [] if maxw == 0 else waits[-maxw:]
                    for w in excess:
                        nop = mybir.InstNoOp(
                            name=self.nc.get_next_instruction_name(),
                            engine=inst.engine, ins=[], outs=[])
                        nop.sync_info = mybir.SyncInfo(on_wait=[w], on_update=[])
                        new.append(nop)
                    inst.sync_info = mybir.SyncInfo(
                        on_wait=keep, on_update=list(si.on_update or []))
                new.append(inst)
            ordered[bb_name] = new
        return super()._lower_ordered_insts(ordered)

    def _drain_and_barrier(self, tick_clock, wait_clock):
        gc = tick_clock.global_clock
        n = len(gc)
        for p in range(n):
            if gc[p] > 0:
                vc = VectorClock([gc[i] if i == p else 0 for i in range(n)])
                nop = self.nc.sync.nop(nofuse=True)
                wait_clock.add_sem_waits(nop.ins, ScopedClock({None: vc}))
        self.nc.sync.drain()
        self.nc.all_engine_barrier()
        assert self.sems is not None
        popped = self.nc._tile_sem_poison_stack.pop()
        assert popped is self._sem_poison
        self.nc.clear_and_free_semaphores(list(self.sems.allocated().values()))
        self.nc.all_engine_barrier()


# ---------------------------------------------------------------- program
def build_program(n_chunks=NCH):
    nc = bass.Bass()
    d_hT = nc.declare_dram_parameter("hT", [DM, SLEN], BF16, isOutput=False)
    d_hres = nc.declare_dram_parameter("hres", [SLEN, DM], BF16, isOutput=False)
    d_w = nc.declare_dram_parameter("wqkv", [DM, 3088], BF16, isOutput=False)
    d_wo = nc.declare_dram_parameter("woT", [DM, DM], BF16, isOutput=False)
    d_lng = nc.declare_dram_parameter("lng", [128, DM], BF16, isOutput=False)
    d_lnb = nc.declare_dram_parameter("lnb", [128, DM], BF16, isOutput=False)
    d_mUI = nc.declare_dram_parameter("maskUI", [128, 128], BF16, isOutput=False)
    d_mUS = nc.declare_dram_parameter("maskUS", [128, 128], BF16, isOutput=False)
    d_out = nc.declare_dram_parameter("out", [SLEN, DM], BF16, isOutput=True)

    with PatchedTileContext(nc) as tc, ExitStack() as ctx:
        P = lambda name, bufs, **kw: ctx.enter_context(
            tc.tile_pool(name=name, bufs=bufs, **kw))
        const = P("const", 1)
        state = P("state", 1)
        hts_p = P("hts", 2)
        xp_p = P("xp", 1)
        f_p = P("f", 2)
        G_p = P("G", 2)
        V_p = P("V", 1)
        t1_p = P("t1", 2)
        cols_p = P("cols", 2)
        sa_p = P("sa", 2)
        sh_p = P("sh", 2)
        nt_p = P("nt", 2)
        z_p = P("z", 1)
        mg_p = P("mg", 2)
        oc_p = P("oc", 1)
        oT_p = P("oT", 1)
        hr_p = P("hr", 2)
        x_p = P("x", 2)
        psP_p = P("psP", 2, space="PSUM")
        psG_p = P("psG", 3, space="PSUM")
        psS_p = P("psS", 3, space="PSUM")

        # ---- constants
        t_mUI = const.tile([128, 128], BF16, tag="mUI", name="mUI")
        t_mUS = const.tile([128, 128], BF16, tag="mUS", name="mUS")
        t_lng = const.tile([128, DM], BF16, tag="lng", name="lng")
        t_lnb = const.tile([128, DM], BF16, tag="lnb", name="lnb")
        t_ones = const.tile([128, 1], BF16, tag="ones", name="ones")
        nc.vector.memset(t_ones[:], 1.0)
        t_w = [const.tile([128, 3088], BF16, tag=f"w{mc}", name=f"w{mc}")
               for mc in range(8)]
        t_wo = [const.tile([128, DM], BF16, tag=f"wo{ic}", name=f"wo{ic}")
                for ic in range(8)]

        def load_consts():
            for mc in range(8):
                nc.sync.dma_start(t_w[mc][:, 0:2048],
                                  d_w[mc * 128:(mc + 1) * 128, 0:2048])
            for mc in range(8):
                nc.sync.dma_start(t_w[mc][:, 2048:3088],
                                  d_w[mc * 128:(mc + 1) * 128, 2048:3088])
            nc.sync.dma_start(t_mUI[:], d_mUI[:])
            nc.sync.dma_start(t_mUS[:], d_mUS[:])
            for ic in range(8):
                nc.sync.dma_start(t_wo[ic][:], d_wo[ic * 128:(ic + 1) * 128, :])
            nc.sync.dma_start(t_lng[:], d_lng[:])
            nc.sync.dma_start(t_lnb[:], d_lnb[:])

        # ---- state
        t_Wm = state.tile([128, NH * 128], F32, tag="Wm", name="Wm")
        nc.vector.memset(t_Wm[:], 0.0)
        t_Wb = state.tile([128, NH * 128], BF16, tag="Wb", name="Wb")
        nc.vector.memset(t_Wb[:], 0.0)
        t_r, t_rb = [], []
        for dc in range(2):
            r = state.tile([128, NH], F32, tag=f"r{dc}", name=f"r{dc}")
            nc.vector.memset(r[:], 0.0)
            t_r.append(r)
            rb = state.tile([128, NH], BF16, tag=f"rb{dc}", name=f"rb{dc}")
            nc.vector.memset(rb[:], 0.0)
            t_rb.append(rb)
        # Y with per-head stride 65; col 64 stays 1.0 so the pW matmuls also
        # produce the per-chunk r increment (sum_t K[t,f]) for free.
        t_Y = state.tile([128, NH * 65], BF16, tag="Y65", name="Y65")
        nc.vector.memset(t_Y[:], 1.0)

        H = {}

        def prepA_dma(c):
            cs = slice(c * 128, (c + 1) * 128)
            hts = hts_p.tile([128, 1024], BF16, tag="hts", name="hts")
            nc.sync.dma_start(
                hts[:].rearrange("p (mc t) -> p mc t", t=128),
                d_hT[:, cs].rearrange("(mc p) t -> p mc t", p=128))
            hr = hr_p.tile([128, DM], BF16, tag="hr", name="hr")
            nc.sync.dma_start(hr[:], d_hres[cs, :])
            return hts, hr

        def prepA(c, pre=None):
            hts, hr = pre if pre is not None else prepA_dma(c)
            # xpC: per head [relu(Q) 64 | relu(-Q) 64 | relu(K) 64 | relu(-K) 64]
            xpC = xp_p.tile([128, 4096], BF16, tag="xpC", name="xpC")
            xr = xpC[:].rearrange("p (h w) -> p h w", w=256)
            Vall = V_p.tile([128, 1024], BF16, tag="V", name="V")
            sig = cols_p.tile([128, NH], F32, tag="sig", name="sig")
            for og in range(6):
                ps = psP_p.tile([128, 512], F32, tag="pp", name="pp")
                for mc in range(8):
                    nc.tensor.matmul(ps[:], hts[:, mc * 128:(mc + 1) * 128],
                                     t_w[mc][:, og * 512:(og + 1) * 512],
                                     start=(mc == 0), stop=(mc == 7))
                psr = ps[:].rearrange("p (h v) -> p h v", v=64)
                if og < 4:      # K (og 0,1) / Q (og 2,3), heads 8*(og%2)..
                    o = 128 if og < 2 else 0
                    hs = slice((og % 2) * 8, (og % 2) * 8 + 8)
                    nc.scalar.activation(xr[:, hs, o:o + 64], psr, AF.Relu)
                    nc.scalar.activation(xr[:, hs, o + 64:o + 128], psr,
                                         AF.Relu, scale=-1.0)
                else:           # V
                    gv = og - 4
                    nc.scalar.copy(Vall[:, gv * 512:(gv + 1) * 512], ps[:])
            psb = psP_p.tile([128, NH], F32, tag="pp", name="pp")
            for mc in range(8):
                nc.tensor.matmul(psb[:], hts[:, mc * 128:(mc + 1) * 128],
                                 t_w[mc][:, 3072:3088],
                                 start=(mc == 0), stop=(mc == 7))
            nc.scalar.activation(sig[:], psb[:], AF.Sigmoid)
            H[c] = dict(xpC=xpC, Vall=Vall, sig=sig, hr=hr)

        def prepB1(c):
            h = H[c]
            # f tiles: per dc, per head [Q-roll-dc 128 | K-roll-dc 128]
            f = [f_p.tile([128, 4096], BF16, tag=f"f{dc}", name=f"f{dc}")
                 for dc in range(2)]
            fr = [f[dc][:].rearrange("p (h w) -> p h w", w=256)
                  for dc in range(2)]
            xpC = h["xpC"]
            xr = xpC[:].rearrange("p (h w) -> p h w", w=256)
            for dc in range(2):
                rl = dc + 1
                for o in (0, 128):  # Q block, K block per head
                    nc.vector.tensor_mul(fr[dc][:, :, o + rl:o + 128],
                                         xr[:, :, o + rl:o + 128],
                                         xr[:, :, o:o + 128 - rl])
                    nc.vector.tensor_mul(fr[dc][:, :, o:o + rl],
                                         xr[:, :, o:o + rl],
                                         xr[:, :, o + 128 - rl:o + 128])
            # feature sums: packed-2x bf16 pair-adds fold 256 values down to
            # 64 per head, then one small fp32 reduce
            frec = []
            for side in range(2):
                o = side * 128
                b1 = V_p.tile([128, 1024], BF16, tag="b1", name="b1")
                b1r = b1[:].rearrange("p (h w) -> p h w", w=64)
                nc.vector.tensor_add(b1r, fr[0][:, :, o:o + 64],
                                     fr[0][:, :, o + 64:o + 128])
                nc.vector.tensor_add(b1r, fr[1][:, :, o:o + 64], b1r)
                nc.vector.tensor_add(b1r, fr[1][:, :, o + 64:o + 128], b1r)
                fs = cols_p.tile([128, NH], F32, tag=f"fs{side}", name=f"fs{side}")
                nc.vector.tensor_reduce(fs[:], b1r, AX.X, OP.add)
                fc = cols_p.tile([128, NH], F32, tag=f"fc{side}", name=f"fc{side}")
                nc.vector.reciprocal(fc[:], fs[:])
                frec.append(fc)
            frecQ, frecK = frec[0], frec[1]
            # materialize the frecK broadcast on Act so the DVE muls run packed
            fkb = V_p.tile([128, NH * 128], BF16, tag="fkb", name="fkb")
            nc.scalar.copy(fkb[:].rearrange("p (h w) -> p h w", w=128),
                           frecK[:].unsqueeze(2).broadcast_to((128, NH, 128)))
            fkr = fkb[:].rearrange("p (h w) -> p h w", w=128)
            for dc in range(2):
                nc.vector.tensor_tensor(
                    fr[dc][:, :, 128:256], fr[dc][:, :, 128:256], fkr, OP.mult)
            # one DMA transpose per dc: G = f^T, per head [Q1T 128 | K1T 128]
            G = [G_p.tile([128, 4096], BF16, tag=f"G{dc}", name=f"G{dc}")
                 for dc in range(2)]
            for dc in range(2):
                nc.sync.dma_start_transpose(
                    G[dc][:].rearrange("p (b l) -> p b l", l=128), f[dc][:])
            h.update(G=G, f=f, frecQ=frecQ)

        def prepB2(c):
            h = H[c]
            G, f, frecQ = h["G"], h["f"], h["frecQ"]
            # grams per head: psum [S1 128 | A 128 | kd 1 | dn 1]
            kd = cols_p.tile([128, NH], F32, tag="kd", name="kd")
            dna = cols_p.tile([128, NH], F32, tag="dna", name="dna")
            Shat = sh_p.tile([128, NH * 128], BF16, tag="sh", name="sh")
            NT = nt_p.tile([128, NH * 128], BF16, tag="nt", name="nt")

            pend = []

            def gram_front(hd):
                ps = psG_p.tile([128, 258], F32, tag="gg", name="gg")
                b = hd * 256
                for dc in range(2):
                    nc.tensor.matmul(ps[:, 0:256], G[dc][:, b + 128:b + 256],
                                     G[dc][:, b:b + 256],
                                     start=(dc == 0), stop=(dc == 1))
                sa = sa_p.tile([128, 256], BF16, tag="sa", name="sa")
                nc.scalar.copy(sa[:], ps[:, 0:256])
                nc.gpsimd.tensor_mul(Shat[:, hd * 128:(hd + 1) * 128],
                                     sa[:, 0:128], t_mUI[:])
                nc.gpsimd.tensor_mul(NT[:, hd * 128:(hd + 1) * 128],
                                     sa[:, 128:256], t_mUS[:])
                return ps

            def gram_back(hd, ps):
                b = hd * 256
                for dc in range(2):
                    nc.tensor.matmul(ps[:, 256:257], G[dc][:, b + 128:b + 256],
                                     t_rb[dc][:, hd:hd + 1],
                                     start=(dc == 0), stop=False)
                nc.tensor.matmul(ps[:, 256:257], NT[:, hd * 128:(hd + 1) * 128],
                                 t_ones[:], start=False, stop=True)
                for dc in range(2):
                    nc.tensor.matmul(ps[:, 257:258], G[dc][:, b:b + 128],
                                     t_rb[dc][:, hd:hd + 1],
                                     start=(dc == 0), stop=False)
                nc.tensor.matmul(ps[:, 257:258], Shat[:, hd * 128:(hd + 1) * 128],
                                 t_ones[:], start=False, stop=True)
                nc.vector.tensor_copy(kd[:, hd:hd + 1], ps[:, 256:257])
                nc.vector.tensor_copy(dna[:, hd:hd + 1], ps[:, 257:258])

            LAG = 2
            for hd in range(NH):
                pend.append((hd, gram_front(hd)))
                if hd >= LAG:
                    gram_back(*pend[hd - LAG])
            for i in range(NH - LAG, NH):
                gram_back(*pend[i])
            if c == 0:
                nc.vector.memset(kd[0:1, :], 1.0)

            # chunk-level columns
            ceps = cols_p.tile([128, NH], F32, tag="ceps", name="ceps")
            nc.vector.tensor_scalar_add(ceps[:], kd[:], EPS)
            cc = cols_p.tile([128, NH], F32, tag="cc", name="cc")
            nc.vector.reciprocal(cc[:], ceps[:])
            t0 = cols_p.tile([128, NH], F32, tag="t0", name="t0")
            nc.vector.tensor_mul(t0[:], kd[:], cc[:])
            cb = cols_p.tile([128, NH], F32, tag="cb", name="cb")
            nc.vector.tensor_mul(cb[:], t0[:], h["sig"][:])
            cbc = cols_p.tile([128, NH], F32, tag="cbc", name="cbc")
            nc.vector.tensor_mul(cbc[:], cb[:], cc[:])
            dnm = cols_p.tile([128, NH], F32, tag="dnm", name="dnm")
            nc.vector.tensor_mul(dnm[:], dna[:], frecQ[:])
            dne = cols_p.tile([128, NH], F32, tag="dne", name="dne")
            nc.vector.tensor_scalar_add(dne[:], dnm[:], EPS)
            dnr = cols_p.tile([128, NH], F32, tag="dnr", name="dnr")
            nc.vector.reciprocal(dnr[:], dne[:])
            dnr2 = cols_p.tile([128, NH], F32, tag="dnr2", name="dnr2")
            nc.vector.tensor_scalar_mul(dnr2[:], dnr[:], SCALE)
            dnrS = cols_p.tile([128, NH], F32, tag="dnrS", name="dnrS")
            nc.vector.tensor_mul(dnrS[:], dnr2[:], frecQ[:])
            # t1 = cb * V (cb broadcast materialized on Act for packed DVE mul)
            cbb = V_p.tile([128, 1024], BF16, tag="cbb", name="cbb")
            nc.scalar.copy(cbb[:].rearrange("p (h v) -> p h v", v=64),
                           cb[:].unsqueeze(2).broadcast_to((128, NH, 64)))
            t1 = t1_p.tile([128, 1024], BF16, tag="t1", name="t1")
            nc.vector.tensor_mul(t1[:], h["Vall"][:], cbb[:])
            h.update(G=G, f=f, Shat=Shat, NT=NT, cbc=cbc, dnrS=dnrS, t1=t1)

        def bscale(out, ps, colvec, g):
            """out (bf16) = psum [128,512] * per-head column broadcast."""
            nc.vector.tensor_tensor(
                out.rearrange("p (h v) -> p h v", v=64),
                ps[:].rearrange("p (h v) -> p h v", v=64),
                colvec[:, g * 8:(g + 1) * 8].unsqueeze(2).broadcast_to((128, 8, 64)),
                OP.mult)

        def scan(c):
            cs = slice(c * 128, (c + 1) * 128)
            h = H.pop(c)
            G, f, cbc, dnrS = h["G"], h["f"], h["cbc"], h["dnrS"]
            # ---- KW + X0 = t1 - cbc*KW (in place over t1)
            X0 = h["t1"]
            for g in range(2):
                ps = psS_p.tile([128, 512], F32, tag="ss", name="ss")
                for j in range(8):
                    hd = g * 8 + j
                    b = hd * 256
                    for dc in range(2):
                        nc.tensor.matmul(ps[:, j * 64:(j + 1) * 64],
                                         G[dc][:, b + 128:b + 256],
                                         t_Wb[:, hd * 128 + dc * 64:
                                              hd * 128 + (dc + 1) * 64],
                                         start=(dc == 0), stop=(dc == 1))
                mg = mg_p.tile([128, 512], BF16, tag="mg", name="mg")
                bscale(mg[:], ps, cbc, g)
                nc.vector.tensor_sub(X0[:, g * 512:(g + 1) * 512],
                                     X0[:, g * 512:(g + 1) * 512], mg[:])
            # ---- Horner iterations: Z <- X0 - cbc*(NT^T Z); last writes t_Y
            y65 = t_Y[:].rearrange("p (h w) -> p h w", w=65)
            Zin = X0
            for it in range(NHORN):
                last = (it == NHORN - 1)
                Zout = None if last else z_p.tile([128, 1024], BF16,
                                                  tag=f"z{it}", name=f"z{it}")
                for g in range(2):
                    ps = psS_p.tile([128, 512], F32, tag="ss", name="ss")
                    for j in range(8):
                        hd = g * 8 + j
                        nc.tensor.matmul(ps[:, j * 64:(j + 1) * 64],
                                         h["NT"][:, hd * 128:(hd + 1) * 128],
                                         Zin[:, hd * 64:(hd + 1) * 64],
                                         start=True, stop=True)
                    mg = mg_p.tile([128, 512], BF16, tag="mg", name="mg")
                    bscale(mg[:], ps, cbc, g)
                    if last:
                        nc.vector.tensor_sub(
                            y65[:, g * 8:(g + 1) * 8, 0:64],
                            X0[:, g * 512:(g + 1) * 512].rearrange(
                                "p (h v) -> p h v", v=64),
                            mg[:].rearrange("p (h v) -> p h v", v=64))
                    else:
                        nc.vector.tensor_sub(Zout[:, g * 512:(g + 1) * 512],
                                             X0[:, g * 512:(g + 1) * 512], mg[:])
                Zin = Zout
            # ---- outc = (QW + tril_incl(S1) Y) * dnrS
            outc = oc_p.tile([128, 1024], BF16, tag="oc", name="oc")
            for g in range(2):
                ps = psS_p.tile([128, 512], F32, tag="ss", name="ss")
                for j in range(8):
                    hd = g * 8 + j
                    b = hd * 256
                    for dc in range(2):
                        nc.tensor.matmul(ps[:, j * 64:(j + 1) * 64],
                                         G[dc][:, b:b + 128],
                                         t_Wb[:, hd * 128 + dc * 64:
                                              hd * 128 + (dc + 1) * 64],
                                         start=(dc == 0), stop=False)
                    nc.tensor.matmul(ps[:, j * 64:(j + 1) * 64],
                                     h["Shat"][:, hd * 128:(hd + 1) * 128],
                                     t_Y[:, hd * 65:hd * 65 + 64],
                                     start=False, stop=True)
                bscale(outc[:, g * 512:(g + 1) * 512], ps, dnrS, g)
            # ---- transpose outc -> outT [i, t] via one DMA
            outT = oT_p.tile([128, 1024], BF16, tag="oT", name="oT")
            nc.sync.dma_start_transpose(
                outT[:].rearrange("p (b l) -> p b l", l=128), outc[:])
            # ---- W state update: Wm += K1^T [Y|1] (col 64 of each 65-block
            # gives the r increment); Wb = bf16(Wm)
            for rr in range(6):
                hds = list(range(3 * rr, min(3 * rr + 3, NH)))
                nh_r = len(hds)
                ps = psS_p.tile([128, nh_r * 130], F32, tag="ss", name="ss")
                for j, hd in enumerate(hds):
                    for dc in range(2):
                        nc.tensor.matmul(
                            ps[:, j * 130 + dc * 65:j * 130 + dc * 65 + 65],
                            f[dc][:, hd * 256 + 128:hd * 256 + 256],
                            t_Y[:, hd * 65:(hd + 1) * 65], start=True, stop=True)
                nc.vector.tensor_add(
                    t_Wm[:, hds[0] * 128:(hds[-1] + 1) * 128].rearrange(
                        "p (j d v) -> p j d v", d=2, v=64),
                    ps[:].rearrange("p (j d w) -> p j d w", d=2, w=65)[
                        :, :, :, 0:64],
                    t_Wm[:, hds[0] * 128:(hds[-1] + 1) * 128].rearrange(
                        "p (j d v) -> p j d v", d=2, v=64))
                for dc in range(2):
                    nc.vector.tensor_add(
                        t_r[dc][:, hds[0]:hds[-1] + 1],
                        ps[:].rearrange("p (j w) -> p j w", w=130)[
                            :, :, dc * 65 + 64:dc * 65 + 65].squeeze(2),
                        t_r[dc][:, hds[0]:hds[-1] + 1])
                nc.scalar.copy(t_Wb[:, hds[0] * 128:(hds[-1] + 1) * 128],
                               t_Wm[:, hds[0] * 128:(hds[-1] + 1) * 128])
            for dc in range(2):
                nc.scalar.copy(t_rb[dc][:], t_r[dc][:])
            # ---- output projection + residual + LN
            x = x_p.tile([128, DM], BF16, tag="x", name="x")
            for og in range(2):
                ps = psP_p.tile([128, 512], F32, tag="pp", name="pp")
                for ic in range(8):
                    nc.tensor.matmul(ps[:], outT[:, ic * 128:(ic + 1) * 128],
                                     t_wo[ic][:, og * 512:(og + 1) * 512],
                                     start=(ic == 0), stop=(ic == 7))
                nc.vector.tensor_add(x[:, og * 512:(og + 1) * 512], ps[:],
                                     h["hr"][:, og * 512:(og + 1) * 512])
            stats = cols_p.tile([128, 2, 6], F32, tag="stats", name="stats")
            for sg in range(2):
                nc.vector.bn_stats(stats[:, sg, :], x[:, sg * 512:(sg + 1) * 512])
            mv = cols_p.tile([128, 2], F32, tag="mv", name="mv")
            nc.vector.bn_aggr(mv[:], stats[:])
            nmu = cols_p.tile([128, 1], F32, tag="nmu", name="nmu")
            nc.vector.tensor_scalar_mul(nmu[:], mv[:, 0:1], -1.0)
            nc.vector.tensor_scalar_add(x[:], x[:], nmu[:])
            vare = cols_p.tile([128, 1], F32, tag="vare", name="vare")
            nc.vector.tensor_scalar(vare[:], mv[:, 1:2], 1.0, float(LN_EPS),
                                    OP.mult, OP.add)
            sd = cols_p.tile([128, 1], F32, tag="sd", name="sd")
            nc.scalar.sqrt(sd[:], vare[:])
            rstd = cols_p.tile([128, 1], F32, tag="rstd", name="rstd")
            nc.vector.reciprocal(rstd[:], sd[:])
            nc.vector.scalar_tensor_tensor(x[:], x[:], rstd[:], t_lng[:],
                                           OP.mult, OP.mult)
            nc.vector.tensor_add(x[:], x[:], t_lnb[:])
            nc.sync.dma_start(d_out[cs, :], x[:])

        order = os.environ.get("KORDER", "ii")
        pre0 = prepA_dma(0)
        load_consts()
        prepA(0, pre0)
        prepB1(0)
        prepB2(0)
        for c in range(n_chunks):
            if c + 1 < n_chunks:
                prepA(c + 1)
                if order == "i":
                    prepB1(c + 1)
            scan(c)
            if c + 1 < n_chunks:
                if order != "i":
                    prepB1(c + 1)
                prepB2(c + 1)

    return nc


# ---------------------------------------------------------------- host side
def _prep_core_inputs(h_b, W_qkvb, W_o, ln_g, ln_b):
    bf16 = ml_dtypes.bfloat16
    hT = np.ascontiguousarray(h_b.T).astype(bf16)                  # [1024, 2048]
    Wr = W_qkvb.reshape(NH, 193, DM)
    wq = np.empty((DM, 3088), dtype=bf16)
    wq[:, 0:1024] = Wr[:, 64:128, :].reshape(1024, DM).T           # K
    wq[:, 1024:2048] = Wr[:, 0:64, :].reshape(1024, DM).T          # Q
    wq[:, 2048:3072] = Wr[:, 128:192, :].reshape(1024, DM).T       # V
    wq[:, 3072:3088] = Wr[:, 192, :].T                             # b
    woT = np.ascontiguousarray(W_o.T).astype(bf16)                 # [i, o]
    lng = np.broadcast_to(ln_g[None, :], (128, DM)).astype(bf16).copy()
    lnb = np.broadcast_to(ln_b[None, :], (128, DM)).astype(bf16).copy()
    ii, jj = np.indices((128, 128))
    mUI = (jj >= ii).astype(bf16)
    mUS = (jj > ii).astype(bf16)
    return {"hT": hT, "hres": h_b.astype(bf16),
            "wqkv": wq, "woT": woT, "lng": lng, "lnb": lnb,
            "maskUI": mUI, "maskUS": mUS}


_cached = {}


def kernel(h, W_qkvb, W_o, ln_g, ln_b):
    h = np.asarray(h, np.float32)
    W_qkvb = np.asarray(W_qkvb, np.float32)
    W_o = np.asarray(W_o, np.float32)
    ln_g = np.asarray(ln_g, np.float32)
    ln_b = np.asarray(ln_b, np.float32)
    if "nc" not in _cached:
        _cached["nc"] = build_program()
    nc = _cached["nc"]
    in_maps = [_prep_core_inputs(h[:, b, :], W_qkvb, W_o, ln_g, ln_b)
               for b in range(BSZ)]
    res = run_bass_kernel_spmd(nc, in_maps, list(range(BSZ)),
                               trace=os.environ.get("BASS_TRACE", "") == "1")
    out = np.stack([res.results[b]["out"] for b in range(BSZ)], axis=1)
    kernel.last_exec_time_ns = res.exec_time_ns
    return out.astype(np.float32)


# revision 56
# speedup vs baseline: 111.0181x; 1.0403x over previous
"""Trainium2 Bass kernel for the CudaNorm FastWeight DPFP transformer layer.

Sharding: batch (8) across the 8 cores; each core runs its batch's full layer.

v3: head-batched DVE/Act ops via 3D strided APs, [K|Q|V|b] projection layout,
interleaved per-head [Q|K] feature tiles so one SBUF->SBUF DMA transpose per
dc produces the whole feature-major G tile, kd/dn via PE column-sum matmuls
accumulated onto K.r/Q.r psum groups, unnormalized-Q trick, depth-1 Horner
Neumann solve with batched cbc post-scaling, stage-major scan with 8-head
psum packing, prepA/scan/prepB software pipelining across chunks.
"""
import os
import numpy as np
import ml_dtypes

import concourse.bass as bass
import concourse.mybir as mybir
from concourse.bass_utils import run_bass_kernel_spmd
from concourse.tile import TileContext
from concourse.vector_clock import ScopedClock, VectorClock
from contextlib import ExitStack

F32 = mybir.dt.float32
BF16 = mybir.dt.bfloat16
AF = mybir.ActivationFunctionType
OP = mybir.AluOpType
AX = mybir.AxisListType

SLEN, BSZ, DM = 2048, 8, 1024
NH, DH, NROLL = 16, 64, 2
D = 2 * NROLL * DH            # 256 feature dim
C = 128                       # chunk length
NCH = SLEN // C               # 16 chunks
EPS, LN_EPS = 1e-5, 1e-5
SCALE = 1.0 / float(np.sqrt(DH))
NHORN = 1                     # Horner/Neumann solve iterations

# ---------------------------------------------------------------- tile ctx
MAXW = 2


class PatchedTileContext(TileContext):
    """Work around walrus TPB sync-command limits: each instruction carries at
    most 2 sync commands (waits+updates); hoist excess waits onto preceding
    same-engine NoOps (1 wait each), and emit the kernel-tail drain's waits
    one-per-nop on SP."""

    def _lower_ordered_insts(self, ordered):
        for bb_name in list(ordered.keys()):
            new = []
            for inst in ordered[bb_name]:
                si = inst.sync_info
                nupd = len(si.on_update) if si is not None and si.on_update else 0
                maxw = max(0, MAXW - nupd)
                if si is not None and si.on_wait and len(si.on_wait) > maxw:
                    waits = list(si.on_wait)
                    excess = waits if maxw == 0 else waits[:-maxw]
                    keep = [] if maxw == 0 else waits[-maxw:]
                    for w in excess:
                        nop = mybir.InstNoOp(
                            name=self.nc.get_next_instruction_name(),
                            engine=inst.engine, ins=[], outs=[])
                        nop.sync_info = mybir.SyncInfo(on_wait=[w], on_update=[])
                        new.append(nop)
                    inst.sync_info = mybir.SyncInfo(
                        on_wait=keep, on_update=list(si.on_update or []))
                new.append(inst)
            ordered[bb_name] = new
        return super()._lower_ordered_insts(ordered)

    def _drain_and_barrier(self, tick_clock, wait_clock):
        gc = tick_clock.global_clock
        n = len(gc)
        for p in range(n):
            if gc[p] > 0:
                vc = VectorClock([gc[i] if i == p else 0 for i in range(n)])
                nop = self.nc.sync.nop(nofuse=True)
                wait_clock.add_sem_waits(nop.ins, ScopedClock({None: vc}))
        self.nc.sync.drain()
        self.nc.all_engine_barrier()
        assert self.sems is not None
        popped = self.nc._tile_sem_poison_stack.pop()
        assert popped is self._sem_poison
        self.nc.clear_and_free_semaphores(list(self.sems.allocated().values()))
        self.nc.all_engine_barrier()


# ---------------------------------------------------------------- program
def build_program(n_chunks=NCH):
    nc = bass.Bass()
    d_hT = nc.declare_dram_parameter("hT", [DM, SLEN], BF16, isOutput=False)
    d_hres = nc.declare_dram_parameter("hres", [SLEN, DM], BF16, isOutput=False)
    d_w = nc.declare_dram_parameter("wqkv", [DM, 3088], BF16, isOutput=False)
    d_wo = nc.declare_dram_parameter("woT", [DM, DM], BF16, isOutput=False)
    d_lng = nc.declare_dram_parameter("lng", [128, DM], BF16, isOutput=False)
    d_lnb = nc.declare_dram_parameter("lnb", [128, DM], BF16, isOutput=False)
    d_mUI = nc.declare_dram_parameter("maskUI", [128, 128], BF16, isOutput=False)
    d_mUS = nc.declare_dram_parameter("maskUS", [128, 128], BF16, isOutput=False)
    d_out = nc.declare_dram_parameter("out", [SLEN, DM], BF16, isOutput=True)

    with PatchedTileContext(nc) as tc, ExitStack() as ctx:
        P = lambda name, bufs, **kw: ctx.enter_context(
            tc.tile_pool(name=name, bufs=bufs, **kw))
        const = P("const", 1)
        state = P("state", 1)
        hts_p = P("hts", 2)
        xp_p = P("xp", 1)
        f_p = P("f", 2)
        G_p = P("G", 2)
        V_p = P("V", 1)
        t1_p = P("t1", 2)
        cols_p = P("cols", 2)
        sa_p = P("sa", 2)
        sh_p = P("sh", 2)
        nt_p = P("nt", 2)
        z_p = P("z", 1)
        mg_p = P("mg", 2)
        oc_p = P("oc", 1)
        oT_p = P("oT", 1)
        hr_p = P("hr", 2)
        x_p = P("x", 2)
        psP_p = P("psP", 2, space="PSUM")
        psG_p = P("psG", 2, space="PSUM")
        psS_p = P("psS", 3, space="PSUM")
        psK_p = P("psK", 1, space="PSUM")

        # ---- constants
        t_mUI = const.tile([128, 128], BF16, tag="mUI", name="mUI")
        t_mUS = const.tile([128, 128], BF16, tag="mUS", name="mUS")
        t_lng = const.tile([128, DM], BF16, tag="lng", name="lng")
        t_lnb = const.tile([128, DM], BF16, tag="lnb", name="lnb")
        t_ones = const.tile([128, 1], BF16, tag="ones", name="ones")
        nc.vector.memset(t_ones[:], 1.0)
        t_w = [const.tile([128, 3088], BF16, tag=f"w{mc}", name=f"w{mc}")
               for mc in range(8)]
        t_wo = [const.tile([128, DM], BF16, tag=f"wo{ic}", name=f"wo{ic}")
                for ic in range(8)]

        def load_consts():
            for mc in range(8):
                nc.sync.dma_start(t_w[mc][:, 0:2048],
                                  d_w[mc * 128:(mc + 1) * 128, 0:2048])
            for mc in range(8):
                nc.sync.dma_start(t_w[mc][:, 2048:3088],
                                  d_w[mc * 128:(mc + 1) * 128, 2048:3088])
            nc.sync.dma_start(t_mUI[:], d_mUI[:])
            nc.sync.dma_start(t_mUS[:], d_mUS[:])
            for ic in range(8):
                nc.sync.dma_start(t_wo[ic][:], d_wo[ic * 128:(ic + 1) * 128, :])
            nc.sync.dma_start(t_lng[:], d_lng[:])
            nc.sync.dma_start(t_lnb[:], d_lnb[:])

        # ---- state
        t_Wm = state.tile([128, NH * 128], F32, tag="Wm", name="Wm")
        nc.vector.memset(t_Wm[:], 0.0)
        t_Wb = state.tile([128, NH * 128], BF16, tag="Wb", name="Wb")
        nc.vector.memset(t_Wb[:], 0.0)
        t_r, t_rb = [], []
        for dc in range(2):
            r = state.tile([128, NH], F32, tag=f"r{dc}", name=f"r{dc}")
            nc.vector.memset(r[:], 0.0)
            t_r.append(r)
            rb = state.tile([128, NH], BF16, tag=f"rb{dc}", name=f"rb{dc}")
            nc.vector.memset(rb[:], 0.0)
            t_rb.append(rb)
        # Y with per-head stride 65; col 64 stays 1.0 so the pW matmuls also
        # produce the per-chunk r increment (sum_t K[t,f]) for free.
        t_Y = state.tile([128, NH * 65], BF16, tag="Y65", name="Y65")
        nc.vector.memset(t_Y[:], 1.0)

        H = {}

        def prepA_dma(c):
            cs = slice(c * 128, (c + 1) * 128)
            hts = hts_p.tile([128, 1024], BF16, tag="hts", name="hts")
            nc.sync.dma_start(
                hts[:].rearrange("p (mc t) -> p mc t", t=128),
                d_hT[:, cs].rearrange("(mc p) t -> p mc t", p=128))
            hr = hr_p.tile([128, DM], BF16, tag="hr", name="hr")
            nc.sync.dma_start(hr[:], d_hres[cs, :])
            return hts, hr

        def prepA(c, pre=None):
            hts, hr = pre if pre is not None else prepA_dma(c)
            # xpC: per head [relu(Q) 64 | relu(-Q) 64 | relu(K) 64 | relu(-K) 64]
            xpC = xp_p.tile([128, 4096], BF16, tag="xpC", name="xpC")
            xr = xpC[:].rearrange("p (h w) -> p h w", w=256)
            Vall = V_p.tile([128, 1024], BF16, tag="V", name="V")
            sig = cols_p.tile([128, NH], F32, tag="sig", name="sig")
            for og in range(6):
                ps = psP_p.tile([128, 512], F32, tag="pp", name="pp")
                for mc in range(8):
                    nc.tensor.matmul(ps[:], hts[:, mc * 128:(mc + 1) * 128],
                                     t_w[mc][:, og * 512:(og + 1) * 512],
                                     start=(mc == 0), stop=(mc == 7))
                psr = ps[:].rearrange("p (h v) -> p h v", v=64)
                if og < 4:      # K (og 0,1) / Q (og 2,3), heads 8*(og%2)..
                    o = 128 if og < 2 else 0
                    hs = slice((og % 2) * 8, (og % 2) * 8 + 8)
                    nc.scalar.activation(xr[:, hs, o:o + 64], psr, AF.Relu)
                    nc.scalar.activation(xr[:, hs, o + 64:o + 128], psr,
                                         AF.Relu, scale=-1.0)
                else:           # V
                    gv = og - 4
                    nc.scalar.copy(Vall[:, gv * 512:(gv + 1) * 512], ps[:])
            psb = psP_p.tile([128, NH], F32, tag="pp", name="pp")
            for mc in range(8):
                nc.tensor.matmul(psb[:], hts[:, mc * 128:(mc + 1) * 128],
                                 t_w[mc][:, 3072:3088],
                                 start=(mc == 0), stop=(mc == 7))
            nc.scalar.activation(sig[:], psb[:], AF.Sigmoid)
            H[c] = dict(xpC=xpC, Vall=Vall, sig=sig, hr=hr)

        def prepB1(c):
            h = H[c]
            # f tiles: per dc, per head [Q-roll-dc 128 | K-roll-dc 128]
            f = [f_p.tile([128, 4096], BF16, tag=f"f{dc}", name=f"f{dc}")
                 for dc in range(2)]
            fr = [f[dc][:].rearrange("p (h w) -> p h w", w=256)
                  for dc in range(2)]
            xpC = h["xpC"]
            xr = xpC[:].rearrange("p (h w) -> p h w", w=256)
            for dc in range(2):
                rl = dc + 1
                for o in (0, 128):  # Q block, K block per head
                    nc.vector.tensor_mul(fr[dc][:, :, o + rl:o + 128],
                                         xr[:, :, o + rl:o + 128],
                                         xr[:, :, o:o + 128 - rl])
                    nc.vector.tensor_mul(fr[dc][:, :, o:o + rl],
                                         xr[:, :, o:o + rl],
                                         xr[:, :, o + 128 - rl:o + 128])
            # feature sums: packed-2x bf16 pair-adds fold 256 values down to
            # 64 per head, then one small fp32 reduce
            frec = []
            for side in range(2):
                o = side * 128
                b1 = V_p.tile([128, 1024], BF16, tag="b1", name="b1")
                b1r = b1[:].rearrange("p (h w) -> p h w", w=64)
                nc.vector.tensor_add(b1r, fr[0][:, :, o:o + 64],
                                     fr[0][:, :, o + 64:o + 128])
                nc.vector.tensor_add(b1r, fr[1][:, :, o:o + 64], b1r)
                nc.vector.tensor_add(b1r, fr[1][:, :, o + 64:o + 128], b1r)
                fs = cols_p.tile([128, NH], F32, tag=f"fs{side}", name=f"fs{side}")
                nc.vector.tensor_reduce(fs[:], b1r, AX.X, OP.add)
                fc = cols_p.tile([128, NH], F32, tag=f"fc{side}", name=f"fc{side}")
                nc.vector.reciprocal(fc[:], fs[:])
                frec.append(fc)
            frecQ, frecK = frec[0], frec[1]
            # materialize the frecK broadcast on Act so the DVE muls run packed
            fkb = V_p.tile([128, NH * 128], BF16, tag="fkb", name="fkb")
            nc.scalar.copy(fkb[:].rearrange("p (h w) -> p h w", w=128),
                           frecK[:].unsqueeze(2).broadcast_to((128, NH, 128)))
            fkr = fkb[:].rearrange("p (h w) -> p h w", w=128)
            for dc in range(2):
                nc.vector.tensor_tensor(
                    fr[dc][:, :, 128:256], fr[dc][:, :, 128:256], fkr, OP.mult)
            # one DMA transpose per dc: G = f^T, per head [Q1T 128 | K1T 128]
            G = [G_p.tile([128, 4096], BF16, tag=f"G{dc}", name=f"G{dc}")
                 for dc in range(2)]
            for dc in range(2):
                nc.sync.dma_start_transpose(
                    G[dc][:].rearrange("p (b l) -> p b l", l=128), f[dc][:])
            h.update(G=G, f=f, frecQ=frecQ)

        def prepB2(c):
            h = H[c]
            G, f, frecQ = h["G"], h["f"], h["frecQ"]
            # grams, two heads per psum bank: [S1|A (head a) | S1|A (head b)];
            # kd/dn columns accumulate in one shared bank psKD [kd 16 | dn 16]
            kd = cols_p.tile([128, NH], F32, tag="kd", name="kd")
            psKD = psK_p.tile([128, 32], F32, tag="kdps", name="kdps")
            Shat = sh_p.tile([128, NH * 128], BF16, tag="sh", name="sh")
            NT = nt_p.tile([128, NH * 128], BF16, tag="nt", name="nt")

            def gram_front(pr):
                ps = psG_p.tile([128, 512], F32, tag="gg", name="gg")
                for e in range(2):
                    b = (pr * 2 + e) * 256
                    for dc in range(2):
                        nc.tensor.matmul(ps[:, e * 256:e * 256 + 256],
                                         G[dc][:, b + 128:b + 256],
                                         G[dc][:, b:b + 256],
                                         start=(dc == 0), stop=(dc == 1))
                sa = sa_p.tile([128, 512], BF16, tag="sa", name="sa")
                nc.scalar.copy(sa[:], ps[:])
                sar = sa[:].rearrange("p (e w) -> p e w", w=256)
                nc.gpsimd.tensor_mul(
                    Shat[:, pr * 256:(pr + 1) * 256].rearrange(
                        "p (e w) -> p e w", w=128),
                    sar[:, :, 0:128],
                    t_mUI[:].unsqueeze(1).broadcast_to((128, 2, 128)))
                nc.gpsimd.tensor_mul(
                    NT[:, pr * 256:(pr + 1) * 256].rearrange(
                        "p (e w) -> p e w", w=128),
                    sar[:, :, 128:256],
                    t_mUS[:].unsqueeze(1).broadcast_to((128, 2, 128)))

            def gram_back(pr):
                for e in range(2):
                    hd = pr * 2 + e
                    b = hd * 256
                    for dc in range(2):
                        nc.tensor.matmul(psKD[:, hd:hd + 1],
                                         G[dc][:, b + 128:b + 256],
                                         t_rb[dc][:, hd:hd + 1],
                                         start=(dc == 0), stop=False)
                    nc.tensor.matmul(psKD[:, hd:hd + 1],
                                     NT[:, hd * 128:(hd + 1) * 128],
                                     t_ones[:], start=False, stop=True)
                    for dc in range(2):
                        nc.tensor.matmul(psKD[:, 16 + hd:17 + hd],
                                         G[dc][:, b:b + 128],
                                         t_rb[dc][:, hd:hd + 1],
                                         start=(dc == 0), stop=False)
                    nc.tensor.matmul(psKD[:, 16 + hd:17 + hd],
                                     Shat[:, hd * 128:(hd + 1) * 128],
                                     t_ones[:], start=False, stop=True)

            for pr in range(NH // 2):
                gram_front(pr)
                if pr >= 1:
                    gram_back(pr - 1)
            gram_back(NH // 2 - 1)
            nc.vector.tensor_copy(kd[:], psKD[:, 0:16])
            if c == 0:
                nc.vector.memset(kd[0:1, :], 1.0)

            # chunk-level columns
            ceps = cols_p.tile([128, NH], F32, tag="ceps", name="ceps")
            nc.vector.tensor_scalar_add(ceps[:], kd[:], EPS)
            cc = cols_p.tile([128, NH], F32, tag="cc", name="cc")
            nc.vector.reciprocal(cc[:], ceps[:])
            t0 = cols_p.tile([128, NH], F32, tag="t0", name="t0")
            nc.vector.tensor_mul(t0[:], kd[:], cc[:])
            cb = cols_p.tile([128, NH], F32, tag="cb", name="cb")
            nc.vector.tensor_mul(cb[:], t0[:], h["sig"][:])
            cbc = cols_p.tile([128, NH], F32, tag="cbc", name="cbc")
            nc.vector.tensor_mul(cbc[:], cb[:], cc[:])
            dnm = cols_p.tile([128, NH], F32, tag="dnm", name="dnm")
            nc.vector.tensor_mul(dnm[:], psKD[:, 16:32], frecQ[:])
            dne = cols_p.tile([128, NH], F32, tag="dne", name="dne")
            nc.vector.tensor_scalar_add(dne[:], dnm[:], EPS)
            dnr = cols_p.tile([128, NH], F32, tag="dnr", name="dnr")
            nc.vector.reciprocal(dnr[:], dne[:])
            dnr2 = cols_p.tile([128, NH], F32, tag="dnr2", name="dnr2")
            nc.vector.tensor_scalar_mul(dnr2[:], dnr[:], SCALE)
            dnrS = cols_p.tile([128, NH], F32, tag="dnrS", name="dnrS")
            nc.vector.tensor_mul(dnrS[:], dnr2[:], frecQ[:])
            # t1 = cb * V (cb broadcast materialized on Act for packed DVE mul)
            cbb = V_p.tile([128, 1024], BF16, tag="cbb", name="cbb")
            nc.scalar.copy(cbb[:].rearrange("p (h v) -> p h v", v=64),
                           cb[:].unsqueeze(2).broadcast_to((128, NH, 64)))
            t1 = t1_p.tile([128, 1024], BF16, tag="t1", name="t1")
            nc.vector.tensor_mul(t1[:], h["Vall"][:], cbb[:])
            h.update(G=G, f=f, Shat=Shat, NT=NT, cbc=cbc, dnrS=dnrS, t1=t1)

        def bscale(out, ps, colvec, g):
            """out (bf16) = psum [128,512] * per-head column broadcast."""
            nc.vector.tensor_tensor(
                out.rearrange("p (h v) -> p h v", v=64),
                ps[:].rearrange("p (h v) -> p h v", v=64),
                colvec[:, g * 8:(g + 1) * 8].unsqueeze(2).broadcast_to((128, 8, 64)),
                OP.mult)

        def scan(c):
            cs = slice(c * 128, (c + 1) * 128)
            h = H.pop(c)
            G, f, cbc, dnrS = h["G"], h["f"], h["cbc"], h["dnrS"]
            # ---- KW + X0 = t1 - cbc*KW (in place over t1)
            X0 = h["t1"]
            for g in range(2):
                ps = psS_p.tile([128, 512], F32, tag="ss", name="ss")
                for j in range(8):
                    hd = g * 8 + j
                    b = hd * 256
                    for dc in range(2):
                        nc.tensor.matmul(ps[:, j * 64:(j + 1) * 64],
                                         G[dc][:, b + 128:b + 256],
                                         t_Wb[:, hd * 128 + dc * 64:
                                              hd * 128 + (dc + 1) * 64],
                                         start=(dc == 0), stop=(dc == 1))
                mg = mg_p.tile([128, 512], BF16, tag="mg", name="mg")
                bscale(mg[:], ps, cbc, g)
                nc.vector.tensor_sub(X0[:, g * 512:(g + 1) * 512],
                                     X0[:, g * 512:(g + 1) * 512], mg[:])
            # ---- Horner iterations: Z <- X0 - cbc*(NT^T Z); last writes t_Y
            y65 = t_Y[:].rearrange("p (h w) -> p h w", w=65)
            Zin = X0
            for it in range(NHORN):
                last = (it == NHORN - 1)
                Zout = None if last else z_p.tile([128, 1024], BF16,
                                                  tag=f"z{it}", name=f"z{it}")
                for g in range(2):
                    ps = psS_p.tile([128, 512], F32, tag="ss", name="ss")
                    for j in range(8):
                        hd = g * 8 + j
                        nc.tensor.matmul(ps[:, j * 64:(j + 1) * 64],
                                         h["NT"][:, hd * 128:(hd + 1) * 128],
                                         Zin[:, hd * 64:(hd + 1) * 64],
                                         start=True, stop=True)
                    mg = mg_p.tile([128, 512], BF16, tag="mg", name="mg")
                    bscale(mg[:], ps, cbc, g)
                    if last:
                        nc.vector.tensor_sub(
                            y65[:, g * 8:(g + 1) * 8, 0:64],
                            X0[:, g * 512:(g + 1) * 512].rearrange(
                                "p (h v) -> p h v", v=64),
                            mg[:].rearrange("p (h v) -> p h v", v=64))
                    else:
                        nc.vector.tensor_sub(Zout[:, g * 512:(g + 1) * 512],
                                             X0[:, g * 512:(g + 1) * 512], mg[:])
                Zin = Zout
            # ---- outc = (QW + tril_incl(S1) Y) * dnrS
            outc = oc_p.tile([128, 1024], BF16, tag="oc", name="oc")
            for g in range(2):
                ps = psS_p.tile([128, 512], F32, tag="ss", name="ss")
                for j in range(8):
                    hd = g * 8 + j
                    b = hd * 256
                    for dc in range(2):
                        nc.tensor.matmul(ps[:, j * 64:(j + 1) * 64],
                                         G[dc][:, b:b + 128],
                                         t_Wb[:, hd * 128 + dc * 64:
                                              hd * 128 + (dc + 1) * 64],
                                         start=(dc == 0), stop=False)
                    nc.tensor.matmul(ps[:, j * 64:(j + 1) * 64],
                                     h["Shat"][:, hd * 128:(hd + 1) * 128],
                                     t_Y[:, hd * 65:hd * 65 + 64],
                                     start=False, stop=True)
                bscale(outc[:, g * 512:(g + 1) * 512], ps, dnrS, g)
            # ---- transpose outc -> outT [i, t] via one DMA
            outT = oT_p.tile([128, 1024], BF16, tag="oT", name="oT")
            nc.sync.dma_start_transpose(
                outT[:].rearrange("p (b l) -> p b l", l=128), outc[:])
            # ---- W state update: Wm += K1^T [Y|1] (col 64 of each 65-block
            # gives the r increment); Wb = bf16(Wm)
            for rr in range(6):
                hds = list(range(3 * rr, min(3 * rr + 3, NH)))
                nh_r = len(hds)
                ps = psS_p.tile([128, nh_r * 130], F32, tag="ss", name="ss")
                for j, hd in enumerate(hds):
                    for dc in range(2):
                        nc.tensor.matmul(
                            ps[:, j * 130 + dc * 65:j * 130 + dc * 65 + 65],
                            f[dc][:, hd * 256 + 128:hd * 256 + 256],
                            t_Y[:, hd * 65:(hd + 1) * 65], start=True, stop=True)
                nc.vector.tensor_add(
                    t_Wm[:, hds[0] * 128:(hds[-1] + 1) * 128].rearrange(
                        "p (j d v) -> p j d v", d=2, v=64),
                    ps[:].rearrange("p (j d w) -> p j d w", d=2, w=65)[
                        :, :, :, 0:64],
                    t_Wm[:, hds[0] * 128:(hds[-1] + 1) * 128].rearrange(
                        "p (j d v) -> p j d v", d=2, v=64))
                for dc in range(2):
                    nc.vector.tensor_add(
                        t_r[dc][:, hds[0]:hds[-1] + 1],
                        ps[:].rearrange("p (j w) -> p j w", w=130)[
                            :, :, dc * 65 + 64:dc * 65 + 65].squeeze(2),
                        t_r[dc][:, hds[0]:hds[-1] + 1])
                nc.scalar.copy(t_Wb[:, hds[0] * 128:(hds[-1] + 1) * 128],
                               t_Wm[:, hds[0] * 128:(hds[-1] + 1) * 128])
            for dc in range(2):
                nc.scalar.copy(t_rb[dc][:], t_r[dc][:])
            # ---- output projection + residual + LN
            x = x_p.tile([128, DM], BF16, tag="x", name="x")
            for og in range(2):
                ps = psP_p.tile([128, 512], F32, tag="pp", name="pp")
                for ic in range(8):
                    nc.tensor.matmul(ps[:], outT[:, ic * 128:(ic + 1) * 128],
                                     t_wo[ic][:, og * 512:(og + 1) * 512],
                                     start=(ic == 0), stop=(ic == 7))
                nc.vector.tensor_add(x[:, og * 512:(og + 1) * 512], ps[:],
                                     h["hr"][:, og * 512:(og + 1) * 512])
            xsum = cols_p.tile([128, 1], F32, tag="xsum", name="xsum")
            nc.vector.tensor_reduce(xsum[:], x[:], AX.X, OP.add)
            nmu = cols_p.tile([128, 1], F32, tag="nmu", name="nmu")
            nc.vector.tensor_scalar_mul(nmu[:], xsum[:], -1.0 / DM)
            nc.vector.tensor_scalar_add(x[:], x[:], nmu[:])
            vscr = oc_p.tile([128, DM], BF16, tag="oc", name="vscr")
            var = cols_p.tile([128, 1], F32, tag="var", name="var")
            nc.vector.scalar_tensor_tensor(vscr[:], x[:], 1.0, x[:],
                                           OP.mult, OP.mult, accum_out=var[:])
            vare = cols_p.tile([128, 1], F32, tag="vare", name="vare")
            nc.vector.tensor_scalar(vare[:], var[:], 1.0 / DM, float(LN_EPS),
                                    OP.mult, OP.add)
            sd = cols_p.tile([128, 1], F32, tag="sd", name="sd")
            nc.scalar.sqrt(sd[:], vare[:])
            rstd = cols_p.tile([128, 1], F32, tag="rstd", name="rstd")
            nc.vector.reciprocal(rstd[:], sd[:])
            nc.vector.scalar_tensor_tensor(x[:], x[:], rstd[:], t_lng[:],
                                           OP.mult, OP.mult)
            nc.vector.tensor_add(x[:], x[:], t_lnb[:])
            nc.sync.dma_start(d_out[cs, :], x[:])

        order = os.environ.get("KORDER", "ii")
        pre0 = prepA_dma(0)
        load_consts()
        prepA(0, pre0)
        prepB1(0)
        prepB2(0)
        for c in range(n_chunks):
            if c + 1 < n_chunks:
                prepA(c + 1)
                if order == "i":
                    prepB1(c + 1)
            scan(c)
            if c + 1 < n_chunks:
                if order != "i":
                    prepB1(c + 1)
                prepB2(c + 1)

    return nc


# ---------------------------------------------------------------- host side
def _prep_core_inputs(h_b, W_qkvb, W_o, ln_g, ln_b):
    bf16 = ml_dtypes.bfloat16
    hT = np.ascontiguousarray(h_b.T).astype(bf16)                  # [1024, 2048]
    Wr = W_qkvb.reshape(NH, 193, DM)
    wq = np.empty((DM, 3088), dtype=bf16)
    wq[:, 0:1024] = Wr[:, 64:128, :].reshape(1024, DM).T           # K
    wq[:, 1024:2048] = Wr[:, 0:64, :].reshape(1024, DM).T          # Q
    wq[:, 2048:3072] = Wr[:, 128:192, :].reshape(1024, DM).T       # V
    wq[:, 3072:3088] = Wr[:, 192, :].T                             # b
    woT = np.ascontiguousarray(W_o.T).astype(bf16)                 # [i, o]
    lng = np.broadcast_to(ln_g[None, :], (128, DM)).astype(bf16).copy()
    lnb = np.broadcast_to(ln_b[None, :], (128, DM)).astype(bf16).copy()
    ii, jj = np.indices((128, 128))
    mUI = (jj >= ii).astype(bf16)
    mUS = (jj > ii).astype(bf16)
    return {"hT": hT, "hres": h_b.astype(bf16),
            "wqkv": wq, "woT": woT, "lng": lng, "lnb": lnb,
            "maskUI": mUI, "maskUS": mUS}


_cached = {}


def kernel(h, W_qkvb, W_o, ln_g, ln_b):
    h = np.asarray(h, np.float32)
    W_qkvb = np.asarray(W_qkvb, np.float32)
    W_o = np.asarray(W_o, np.float32)
    ln_g = np.asarray(ln_g, np.float32)
    ln_b = np.asarray(ln_b, np.float32)
    if "nc" not in _cached:
        _cached["nc"] = build_program()
    nc = _cached["nc"]
    in_maps = [_prep_core_inputs(h[:, b, :], W_qkvb, W_o, ln_g, ln_b)
               for b in range(BSZ)]
    res = run_bass_kernel_spmd(nc, in_maps, list(range(BSZ)),
                               trace=os.environ.get("BASS_TRACE", "") == "1")
    out = np.stack([res.results[b]["out"] for b in range(BSZ)], axis=1)
    kernel.last_exec_time_ns = res.exec_time_ns
    return out.astype(np.float32)
